# revision 1
# baseline (speedup 1.0000x reference)
"""Trainium kernel for nn_EpsilonState: batched log-amplitude of Gaussian-state
overlaps.

Math: the reference's per-sample pair of 128x128 Pfaffians reduces exactly to a
pair of 32x32 Pfaffians S_s built elementwise from four shared 32x32 matrices
(Z11, Z10, Z01, Z00) sign-modulated by the sample's sigma = sign(zz) vector:

    S_s = -(sig sig^T) . Z11 + i (sig 1^T) . Z10 + i (1 sig^T) . Z01 + Z00
    amp_s = C_sector * Pf(S_s)          (C_sector from shared host setup)
    out_b = log(amp_m + x_b[31] * amp_p)

Device: 8 cores x 16 matrices (8 samples x 2 sectors), each core holds its 16
matrices as a 4x4 grid of 32x32 blocks inside [128,128] fp32 re/im tiles and
runs 15 batched Parlett-Reid elimination steps; the Pfaffian is the product of
pivots, accumulated per-matrix. Row broadcasts use constant selector matmuls
on the tensor engine; all elementwise work uses broadcast APs on DVE.
"""
import os
import numpy as np

import concourse.bass as bass
from concourse import bacc
import concourse.mybir as mybir
import concourse.tile as tile
from concourse.bass_utils import run_bass_kernel_spmd

f32 = mybir.dt.float32
P = 128
n = 32
N = 64
NCORES = 8
CSCALE = 64.0
SHIFT = -51.0
AOT = mybir.AluOpType

LAST_RESULTS = None  # stash of BassKernelResults for test harness introspection


# ----------------------------------------------------------------------------
# host-side shared setup (float64 numpy; depends only on s0, H1, H2)
# ----------------------------------------------------------------------------

def _slog_pf(A):
    A = A.copy()
    m = A.shape[0]
    sign_val = 1.0 + 0j
    logpf = 0.0
    for i in range(m - 2):
        x_ = A[:, i].copy()
        nidx = i + 1
        ar = np.arange(m)
        xn = x_[nidx]
        x_[ar <= nidx] = 0
        sigma = np.vdot(x_, x_)
        norm_x = np.sqrt(xn.conj() * xn + sigma)
        phase = 1.0 if xn == 0 else xn / np.abs(xn)
        vn = xn + phase * norm_x
        alpha = -phase * norm_x
        v = x_.copy()
        v[nidx] = vn
        if sigma == 0:
            v = np.zeros_like(x_)
            tau = 0
            alpha = xn
        else:
            v = v / np.linalg.norm(v)
            tau = 2
        w = tau * (A @ v.conj())
        A = A + np.outer(v, w) - np.outer(w, v)
        logpf += np.log(np.abs(1 - tau)) + (np.log(np.abs(-alpha)) if i % 2 == 0 else 0.0)
        sign_val *= ((1 - tau) / np.abs(1 - tau)) * ((-alpha / np.abs(-alpha)) if i % 2 == 0 else 1.0)
    logpf += np.log(np.abs(A[m - 2, m - 1]))
    sign_val *= A[m - 2, m - 1] / np.abs(A[m - 2, m - 1])
    return sign_val, logpf


def _gen_v(zz, PX):
    sgn = np.sign(zz).astype(np.float64).copy()
    sgn[-1] = -PX * sgn[-1]
    norm = 1 / np.sqrt(2.0)
    v = np.zeros((N, n), dtype=np.complex128)
    for k in range(n):
        v[2 * k + 1, k] = -1j * sgn[k] * norm
        v[(2 * k + 2) % N, k] = norm
    return v


def _gf2(L, R):
    M = L.conj().T @ R
    X = np.linalg.solve(M, L.conj().T)
    return np.eye(N) - 2 * (R @ X)


def _logeta_g_expH(H):
    Hh = 1j * (H - H.T) / 2
    e, v = np.linalg.eigh(Hh)
    green = np.real(v @ np.diag(1j * np.tan(e / 2)) @ v.conj().T)
    e_pos = e[: N // 2]
    logeta = np.sum(np.log(np.cos(e_pos / 2).astype(np.complex128)))
    expH = v @ np.diag(np.exp(-1j * e)) @ v.conj().T
    return logeta, green, expH


def _plus_state():
    st = np.zeros((N, n), dtype=np.complex128)
    for k in range(n):
        st[2 * k, k] = -1j / np.sqrt(2)
        st[2 * k + 1, k] = 1 / np.sqrt(2)
    return st


def _minus_state():
    st = np.zeros((N, n), dtype=np.complex128)
    for k in range(n):
        st[2 * k, k] = (1j if k == n - 1 else -1j) / np.sqrt(2)
        st[2 * k + 1, k] = 1 / np.sqrt(2)
    return st


def _log_eta_prop(G1, G2, l1, l2):
    A = (G1 - G1.T) * 0.5
    D = (G2 - G2.T) * 0.5
    pfmat = np.block([[A, -np.eye(N)], [np.eye(N), D]])
    sign_pref = (-1) ** (N // 2)
    s, l = _slog_pf(pfmat)
    return l1 + l2 + np.log(sign_pref * s) + l


def _sector_setup(R, Ghz, logeta_Ghz, PX):
    A = (Ghz - Ghz.T) * 0.5
    Ea = np.zeros((N, n))
    Eb = np.zeros((N, n))
    for k in range(n):
        Ea[2 * k + 1, k] = 1 / np.sqrt(2.0)
        Eb[(2 * k + 2) % N, k] = 1 / np.sqrt(2.0)
    m1 = Ea.T @ R
    m0 = Eb.T @ R
    F11 = R.T @ A @ R
    F11inv = np.linalg.inv(F11)
    P1 = m1.T + R.T @ A @ Ea
    P0 = m0.T + R.T @ A @ Eb
    q11 = Ea.T @ A @ Ea
    q12 = Ea.T @ A @ Eb
    q21 = Eb.T @ A @ Ea
    q22 = Eb.T @ A @ Eb
    Z11 = q11 + P1.T @ F11inv @ P1
    Z10 = q12 + P1.T @ F11inv @ P0
    Z01 = q21 + P0.T @ F11inv @ P1
    Z00 = q22 + P0.T @ F11inv @ P0
    Ainv = np.linalg.inv(A)
    sA, lA = _slog_pf(A)
    sAi, lAi = _slog_pf(Ainv)
    sF, lF = _slog_pf(F11)
    # det(M) is the same for every valid sigma (parity constrained); use x=ones
    xr = np.ones(n)
    zzr = xr * np.roll(xr, -1)
    sig = np.sign(zzr)
    sig[-1] *= -PX
    Ls = Ea * (-1j * sig)[None, :] + Eb
    detM = np.linalg.det(Ls.conj().T @ R)
    logC = (logeta_Ghz + np.log(sA) + lA + np.log(sAi) + lAi
            + np.log(sF) + lF - np.log(detM))
    return dict(Z11=Z11, Z10=Z10, Z01=Z01, Z00=Z00, logC=logC)


_setup_cache = {}
_nc_cache = None


def _shared_setup(s0, H1, H2):
    key = (s0.tobytes(), H1.tobytes(), H2.tobytes())
    if key in _setup_cache:
        return _setup_cache[key]
    ps, ms = _plus_state(), _minus_state()
    zz0 = s0 * np.roll(s0, -1)
    v_plus = _gen_v(zz0, 1)
    v_minus = _gen_v(zz0, -1)
    Gz_plus = _gf2(v_plus, v_plus)
    Gz_minus = _gf2(v_minus, v_minus)
    le_p, G_p, expH_p = _logeta_g_expH(H1)
    le_m, G_m, expH_m = _logeta_g_expH(H2)
    Ghz_plus = _gf2(v_plus, expH_p @ v_plus)
    Ghz_minus = _gf2(v_minus, expH_m @ v_minus)
    logeta_Ghz_plus = _log_eta_prop(G_p, Gz_plus, le_p, 0.0)
    logeta_Ghz_minus = _log_eta_prop(G_m, Gz_minus, le_m, 0.0)
    sp = _sector_setup(ps, Ghz_plus, logeta_Ghz_plus, 1)
    sm = _sector_setup(ms, Ghz_minus, logeta_Ghz_minus, -1)
    K_p = np.exp(sp['logC'] - 16 * np.log(CSCALE) - SHIFT)
    K_m = np.exp(sm['logC'] - 16 * np.log(CSCALE) - SHIFT)

    # z-cat planes [8,128,128]: sector by g parity (even g: plus, odd: minus)
    zplanes = np.zeros((8, P, P), np.float32)
    for gi in range(4):
        st = sp if gi % 2 == 0 else sm
        for pl, mat in enumerate([st['Z11'], st['Z10'], st['Z01'], st['Z00']]):
            blk = (mat * CSCALE).astype(np.complex64)
            for mi in range(4):
                zplanes[2 * pl, mi * 32:mi * 32 + 32, gi * 32:gi * 32 + 32] = blk.real
                zplanes[2 * pl + 1, mi * 32:mi * 32 + 32, gi * 32:gi * 32 + 32] = blk.imag

    bones = np.zeros((P, P), np.float32)
    for p1 in range(P):
        bones[p1, (p1 // 32) * 32:(p1 // 32) * 32 + 32] = 1.0
    rmask = np.zeros((P, 31), np.float32)
    for j in range(31):
        rmask[:, j] = (np.arange(P) % 32 == j)
    diagm = np.zeros((P, 32), np.float32)
    for p1 in range(P):
        diagm[p1, p1 % 32] = 1.0
    pmaskm = np.zeros((P, 16), np.float32)
    for s in range(16):
        pmaskm[:, s] = (np.arange(P) % 32 >= 2 * s + 2)

    res = dict(zplanes=zplanes, bones=bones, rmask=rmask, diag=diagm,
               pmask=pmaskm, K_p=K_p, K_m=K_m)
    _setup_cache[key] = res
    return res


# ----------------------------------------------------------------------------
# device program
# ----------------------------------------------------------------------------

def _build_nc(nsteps=15, dump=False):
    global _nc_cache
    if _nc_cache is not None and nsteps == 15 and not dump:
        return _nc_cache
    nc = bacc.Bacc()
    zcat_d = nc.dram_tensor("zcat", [8, P, P], f32, kind="ExternalInput")
    bones_d = nc.dram_tensor("bones", [P, P], f32, kind="ExternalInput")
    rmask_d = nc.dram_tensor("rmask", [P, 31], f32, kind="ExternalInput")
    diag_d = nc.dram_tensor("diag", [P, 32], f32, kind="ExternalInput")
    pmask_d = nc.dram_tensor("pmask", [P, 16], f32, kind="ExternalInput")
    sigc_d = nc.dram_tensor("sigc", [P, 4], f32, kind="ExternalInput")
    sigf_d = nc.dram_tensor("sigf", [P, P], f32, kind="ExternalInput")
    kre_d = nc.dram_tensor("kre", [P, 4], f32, kind="ExternalInput")
    kim_d = nc.dram_tensor("kim", [P, 4], f32, kind="ExternalInput")
    out_d = nc.dram_tensor("out", [P, 4], f32, kind="ExternalOutput")

    with tile.TileContext(nc) as tc:
        with tc.tile_pool(name="const", bufs=1) as cpool, \
             tc.tile_pool(name="state", bufs=1) as spool, \
             tc.tile_pool(name="temps", bufs=2) as tpool, \
             tc.tile_pool(name="psum", bufs=2, space="PSUM") as ppool:

            zc = cpool.tile([P, 8, P], f32, tag="zc")
            for i in range(8):
                nc.sync.dma_start(zc[:, i, :], zcat_d[i])
            bones = cpool.tile([P, P], f32, tag="bones")
            nc.sync.dma_start(bones[:], bones_d[:])
            rmask = cpool.tile([P, 31], f32, tag="rmask")
            nc.sync.dma_start(rmask[:], rmask_d[:])
            diag = cpool.tile([P, 32], f32, tag="diag")
            nc.sync.dma_start(diag[:], diag_d[:])
            pmask = cpool.tile([P, 16], f32, tag="pmask")
            nc.sync.dma_start(pmask[:], pmask_d[:])
            sigc = cpool.tile([P, 4], f32, tag="sigc")
            nc.sync.dma_start(sigc[:], sigc_d[:])
            sigf = cpool.tile([P, P], f32, tag="sigf")
            nc.sync.dma_start(sigf[:], sigf_d[:])
            kre = cpool.tile([P, 4], f32, tag="kre")
            nc.sync.dma_start(kre[:], kre_d[:])
            kim = cpool.tile([P, 4], f32, tag="kim")
            nc.sync.dma_start(kim[:], kim_d[:])

            # selectors built on GpSimd (idle engine): sel_j = bones * rowmask_j
            selt = cpool.tile([P, 31, P], f32, tag="selt")
            for j in range(31):
                nc.scalar.mul(selt[:, j, :], bones[:], rmask[:, j:j + 1])

            # S state: concatenated planes [P, ch(re,im), g, c]
            Scat = spool.tile([P, 2, 4, 32], f32, tag="Scat")
            Sre = Scat[:, 0]
            Sim = Scat[:, 1]
            prod_re = spool.tile([P, 4], f32, tag="prodre")
            prod_im = spool.tile([P, 4], f32, tag="prodim")
            nc.gpsimd.memset(prod_re[:], 1.0)
            nc.gpsimd.memset(prod_im[:], 0.0)

            def z(i):
                return zc[:, i, :].rearrange("p (g c) -> p g c", g=4)

            sigf_v = sigf[:].rearrange("p (g c) -> p g c", g=4)
            sigc_b = sigc[:, :, None].broadcast_to([P, 4, 32])

            # ---- build S = -so*Z11 + i*sig_r*Z10 + i*sig_c*Z01 + Z00 ----
            so = tpool.tile([P, 4, 32], f32, tag="so")
            nc.vector.tensor_mul(so[:], sigf_v, sigc_b)
            t0 = tpool.tile([P, 4, 32], f32, tag="t0")
            nc.vector.tensor_mul(t0[:], so[:], z(0))
            nc.vector.tensor_sub(Sre, z(6), t0[:])
            nc.vector.tensor_mul(t0[:], z(3), sigc_b)
            nc.vector.tensor_sub(Sre, Sre, t0[:])
            nc.vector.tensor_mul(t0[:], z(5), sigf_v)
            nc.vector.tensor_sub(Sre, Sre, t0[:])
            nc.vector.tensor_mul(t0[:], so[:], z(1))
            nc.vector.tensor_sub(Sim, z(7), t0[:])
            nc.vector.tensor_mul(t0[:], z(2), sigc_b)
            nc.vector.tensor_add(Sim, Sim, t0[:])
            nc.vector.tensor_mul(t0[:], z(4), sigf_v)
            nc.vector.tensor_add(Sim, Sim, t0[:])

            Sre_f = Sre.rearrange("p g c -> p (g c)")
            Sim_f = Sim.rearrange("p g c -> p (g c)")

            def pivot_copy(abcat, col):
                """copy pivot (both ch) out of PSUM: pv[p, g, ch]."""
                pv = tpool.tile([P, 4, 2], f32, tag="pv")
                # in: abcat[p, ch, g, col] enumerated (g, ch)
                a0 = abcat[:]
                src = bass.AP(tensor=a0.tensor, offset=a0.offset + col,
                              ap=[a0.ap[0], [32, 4], [4 * 32, 2]])
                nc.vector.tensor_copy(pv[:], src)
                return pv

            def prod_mul(pv):
                # prod *= pivot (GpSimd; off the critical path)
                t1 = tpool.tile([P, 4], f32, tag="pt1")
                t2 = tpool.tile([P, 4], f32, tag="pt2")
                t3 = tpool.tile([P, 4], f32, tag="pt3")
                t4 = tpool.tile([P, 4], f32, tag="pt4")
                pvr = pv[:, :, 0]
                pvi = pv[:, :, 1]
                nc.gpsimd.tensor_mul(t1[:], prod_re[:], pvr)
                nc.gpsimd.tensor_mul(t2[:], prod_im[:], pvi)
                nc.gpsimd.tensor_mul(t3[:], prod_re[:], pvi)
                nc.gpsimd.tensor_mul(t4[:], prod_im[:], pvr)
                nc.gpsimd.tensor_sub(prod_re[:], t1[:], t2[:])
                nc.gpsimd.tensor_add(prod_im[:], t3[:], t4[:])

            diag_b = diag[:, None, None, :].broadcast_to([P, 4, 4, 32])

            for s in range(nsteps):
                k = 2 * s
                # row broadcasts: abcat planes (ar, ai, br, bi) in one PSUM tile
                abcat = ppool.tile([P, 4, 4, 32], f32, tag="abcat")
                Scat_f = Scat[:].rearrange("p ch g c -> p (ch g c)")
                nc.tensor.matmul(
                    abcat[:, 0:2].rearrange("p ch g c -> p (ch g c)"),
                    selt[:, k, :], Scat_f, start=True, stop=True)
                nc.tensor.matmul(
                    abcat[:, 2:4].rearrange("p ch g c -> p (ch g c)"),
                    selt[:, k + 1, :], Scat_f, start=True, stop=True)

                pv = pivot_copy(abcat, k + 1)
                prod_mul(pv)

                # 1/|piv|^2 and inv = piv * rec (conj handled by sign layout)
                sq = tpool.tile([P, 4, 2], f32, tag="sq")
                nc.vector.tensor_mul(sq[:], pv[:], pv[:])
                den = tpool.tile([P, 4], f32, tag="den")
                nc.vector.tensor_reduce(den[:], sq[:], axis=mybir.AxisListType.X,
                                        op=AOT.add)
                rec = tpool.tile([P, 4], f32, tag="rec")
                nc.vector.reciprocal(rec[:], den[:])
                inv = tpool.tile([P, 4, 2], f32, tag="inv")
                nc.vector.tensor_mul(inv[:], pv[:],
                                     rec[:, :, None].broadcast_to([P, 4, 2]))

                # masked-diagonal extraction of all four row vectors at once:
                # X4[p, q, g] = sum_c abcat[p, q, g, c] * diag[p, c]
                w = 32 - (k + 2)
                ext = tpool.tile([P, 4, 4, 32], f32, tag="ext")
                nc.vector.tensor_mul(ext[:, :, :, k + 2:], abcat[:, :, :, k + 2:],
                                     diag_b[:, :, :, k + 2:])
                X4 = tpool.tile([P, 4, 4], f32, tag="X4")
                nc.vector.tensor_reduce(X4[:], ext[:, :, :, k + 2:],
                                        axis=mybir.AxisListType.X, op=AOT.add)
                # X4 q-order: (ac2r, ac2i, bc2r, bc2i) where ac2=-a_r, bc2=-b_r

                # X4sw: (ac2i, -ac2r, bc2i, -bc2r)
                X4sw = tpool.tile([P, 4, 4], f32, tag="X4sw")
                x0 = X4[:]
                xs = X4sw[:]
                src_odd = bass.AP(tensor=x0.tensor, offset=x0.offset + 4,
                                  ap=[x0.ap[0], [8, 2], [1, 4]])
                dst_even = bass.AP(tensor=xs.tensor, offset=xs.offset,
                                   ap=[xs.ap[0], [8, 2], [1, 4]])
                nc.vector.tensor_copy(dst_even, src_odd)
                src_even = bass.AP(tensor=x0.tensor, offset=x0.offset,
                                   ap=[x0.ap[0], [8, 2], [1, 4]])
                dst_odd = bass.AP(tensor=xs.tensor, offset=xs.offset + 4,
                                  ap=[xs.ap[0], [8, 2], [1, 4]])
                nc.vector.tensor_scalar(out=dst_odd, in0=src_even, scalar1=-1.0,
                                        scalar2=None, op0=AOT.mult)

                # UVW = X4*ivr + X4sw*ivi -> (w2r, w2i, u2r, u2i)
                tq1 = tpool.tile([P, 4, 4], f32, tag="tq1")
                tq2 = tpool.tile([P, 4, 4], f32, tag="tq2")
                nc.vector.tensor_mul(
                    tq1[:], X4[:], inv[:, None, :, 0].broadcast_to([P, 4, 4]))
                nc.vector.tensor_mul(
                    tq2[:], X4sw[:], inv[:, None, :, 1].broadcast_to([P, 4, 4]))
                UVW = tpool.tile([P, 4, 4], f32, tag="UVW")
                nc.vector.tensor_add(UVW[:], tq1[:], tq2[:])

                # per-row-plane coefficient pairs (ch-order re,im):
                #  ar: (u2r, u2i)  ai: (-u2i, u2r)  br: (-w2r, -w2i)  bi: (w2i, -w2r)
                vai = tpool.tile([P, 2, 4], f32, tag="vai")
                vbr = tpool.tile([P, 2, 4], f32, tag="vbr")
                vbi = tpool.tile([P, 2, 4], f32, tag="vbi")
                nc.scalar.mul(vai[:, 0], UVW[:, 3], -1.0)
                nc.scalar.copy(vai[:, 1], UVW[:, 2])
                nc.vector.tensor_scalar(out=vbr[:], in0=UVW[:, 0:2], scalar1=-1.0,
                                        scalar2=None, op0=AOT.mult)
                nc.vector.tensor_copy(vbi[:, 0], UVW[:, 1])
                nc.vector.tensor_scalar(out=vbi[:, 1], in0=UVW[:, 0], scalar1=-1.0,
                                        scalar2=None, op0=AOT.mult)
                var = UVW[:, 2:4]  # (u2r, u2i) view

                pt = tpool.tile([P, 2, 4, 32], f32, tag="pt")
                for rp, vc in ((0, var), (1, vai[:]), (2, vbr[:]), (3, vbi[:])):
                    nc.vector.tensor_mul(
                        pt[:, :, :, k + 2:],
                        abcat[:, rp, None, :, k + 2:].broadcast_to([P, 2, 4, w]),
                        vc[:, :, :, None].broadcast_to([P, 2, 4, w]))
                    nc.vector.scalar_tensor_tensor(
                        out=Scat[:, :, :, k + 2:], in0=pt[:, :, :, k + 2:],
                        scalar=1.0, in1=Scat[:, :, :, k + 2:],
                        op0=AOT.mult, op1=AOT.add)

            if dump:
                dump_sre = nc.dram_tensor("dump_sre", [P, P], f32, kind="ExternalOutput")
                dump_sim = nc.dram_tensor("dump_sim", [P, P], f32, kind="ExternalOutput")
                dump_pre = nc.dram_tensor("dump_pre", [P, 4], f32, kind="ExternalOutput")
                dump_pim = nc.dram_tensor("dump_pim", [P, 4], f32, kind="ExternalOutput")
                nc.sync.dma_start(dump_sre[:], Sre_f)
                nc.sync.dma_start(dump_sim[:], Sim_f)
                nc.sync.dma_start(dump_pre[:], prod_re[:])
                nc.sync.dma_start(dump_pim[:], prod_im[:])

            # final pivot: S[30, 31]
            abcat = ppool.tile([P, 4, 4, 32], f32, tag="abcat")
            nc.tensor.matmul(
                abcat[:, 0:2].rearrange("p ch g c -> p (ch g c)"),
                selt[:, 30, :], Scat[:].rearrange("p ch g c -> p (ch g c)"),
                start=True, stop=True)
            pv = pivot_copy(abcat, 31)
            prod_mul(pv)

            # E = K * prod ; pair-sum sectors; emit [re0, im0, re1, im1]
            er = tpool.tile([P, 4], f32, tag="er")
            ei = tpool.tile([P, 4], f32, tag="ei")
            t1 = tpool.tile([P, 4], f32, tag="ft1")
            t2 = tpool.tile([P, 4], f32, tag="ft2")
            nc.vector.tensor_mul(t1[:], prod_re[:], kre[:])
            nc.vector.tensor_mul(t2[:], prod_im[:], kim[:])
            nc.vector.tensor_sub(er[:], t1[:], t2[:])
            nc.vector.tensor_mul(t1[:], prod_re[:], kim[:])
            nc.vector.tensor_mul(t2[:], prod_im[:], kre[:])
            nc.vector.tensor_add(ei[:], t1[:], t2[:])
            outt = tpool.tile([P, 2, 2], f32, tag="outt")
            er_v = er[:].rearrange("p (j t) -> p j t", t=2)
            ei_v = ei[:].rearrange("p (j t) -> p j t", t=2)
            nc.vector.tensor_add(outt[:, :, 0], er_v[:, :, 0], er_v[:, :, 1])
            nc.vector.tensor_add(outt[:, :, 1], ei_v[:, :, 0], ei_v[:, :, 1])
            nc.sync.dma_start(out_d[:], outt[:].rearrange("p j t -> p (j t)"))

    nc.compile()
    if nsteps == 15 and not dump:
        _nc_cache = nc
    return nc


# ----------------------------------------------------------------------------
# entry point
# ----------------------------------------------------------------------------

def kernel(x, s0, H1, H2):
    global LAST_RESULTS
    x64 = np.asarray(x, dtype=np.float64)
    s064 = np.asarray(s0, dtype=np.float64)
    H164 = np.asarray(H1, dtype=np.float64)
    H264 = np.asarray(H2, dtype=np.float64)
    B = x64.shape[0]
    assert B == 64 and x64.shape[1] == n

    st = _shared_setup(s064, H164, H264)
    nc = _build_nc()

    zz = x64 * np.roll(x64, -1, axis=1)          # [64, 32]
    sgn = np.sign(zz)

    in_maps = []
    for c in range(NCORES):
        sigc = np.zeros((P, 4), np.float32)
        sigf = np.zeros((P, P), np.float32)
        kre = np.zeros((P, 4), np.float32)
        kim = np.zeros((P, 4), np.float32)
        for mi in range(4):
            for gi in range(4):
                samp = c * 8 + mi * 2 + gi // 2
                plus = (gi % 2 == 0)
                sig = sgn[samp].copy()
                sig[-1] *= -1.0 if plus else 1.0
                sigc[mi * 32:mi * 32 + 32, gi] = sig
                sigf[mi * 32:mi * 32 + 32, gi * 32:gi * 32 + 32] = sig[None, :]
                K = st['K_p'] * (x64[samp, -1] * s064[-1]) if plus else st['K_m']
                kre[mi * 32:mi * 32 + 32, gi] = np.float32(K.real)
                kim[mi * 32:mi * 32 + 32, gi] = np.float32(K.imag)
        in_maps.append(dict(zcat=st['zplanes'], bones=st['bones'], rmask=st['rmask'],
                            diag=st['diag'], pmask=st['pmask'],
                            sigc=sigc, sigf=sigf, kre=kre, kim=kim))

    trace = bool(int(os.environ.get("PFK_TRACE", "0")))
    res = run_bass_kernel_spmd(nc, in_maps, core_ids=list(range(NCORES)),
                               trace=trace)
    LAST_RESULTS = res

    out = np.zeros(B, dtype=np.complex128)
    for c in range(NCORES):
        o = res.results[c]["out"]
        for mi in range(4):
            for j in range(2):
                zv = complex(o[mi * 32, 2 * j], o[mi * 32, 2 * j + 1])
                out[c * 8 + mi * 2 + j] = np.log(zv) + SHIFT
    return out



# revision 9
# speedup vs baseline: 1.3760x; 1.3760x over previous
"""Trainium kernel for nn_EpsilonState: batched log-amplitude of Gaussian-state
overlaps.

Math: each sample reduces to a pair of 32x32 complex skew Pfaffians S built
elementwise from four shared 32x32 matrices (host-side), sign-modulated by the
sample's sigma vector:

    S = Z00 - (sig sig^T).Z11 + i (sig 1^T).Z10 + i (1 sig^T).Z01   (x CSCALE)
    Pf(S) = prod_s pivots of Parlett-Reid elimination (no pivoting)
    out_b = log(K_m Pf_m + x_b[31] K_p Pf_p) + SHIFT                 (host)

Device: 8 cores x 16 matrices (8 samples x 2 sectors); each core holds its 16
matrices as [128p, 2ch, 4g, 32c] fp32 (4 partition blocks x 4 column groups)
and runs 15 Parlett-Reid steps. Row broadcasts are fp32r selector matmuls on
the tensor engine (exact: weights are 0/1); column vectors are read directly
from S (skew symmetry) so only the pivot needs the broadcast. Pivots are
stored per step and the complex product is taken on the host in float64.
Rank-2 updates are split vector (a-rows) / gpsimd (b-rows, via a scalar-engine
PSUM->SBUF copy since gpsimd has no PSUM port).
"""
import os
import numpy as np

import concourse.bass as bass
from concourse import bacc
import concourse.mybir as mybir
import concourse.tile as tile
from concourse.bass_utils import run_bass_kernel_spmd

f32 = mybir.dt.float32
f32r = mybir.dt.float32r
P = 128
n = 32
N = 64
NCORES = 8
CSCALE = 64.0
SHIFT = -51.0
AOT = mybir.AluOpType

LAST_RESULTS = None  # stash of BassKernelResults for test harness introspection


# ----------------------------------------------------------------------------
# host-side shared setup (float64 numpy; depends only on s0, H1, H2)
# ----------------------------------------------------------------------------

def _slog_pf(A):
    A = A.copy()
    m = A.shape[0]
    sign_val = 1.0 + 0j
    logpf = 0.0
    for i in range(m - 2):
        x_ = A[:, i].copy()
        nidx = i + 1
        ar = np.arange(m)
        xn = x_[nidx]
        x_[ar <= nidx] = 0
        sigma = np.vdot(x_, x_)
        norm_x = np.sqrt(xn.conj() * xn + sigma)
        phase = 1.0 if xn == 0 else xn / np.abs(xn)
        vn = xn + phase * norm_x
        alpha = -phase * norm_x
        v = x_.copy()
        v[nidx] = vn
        if sigma == 0:
            v = np.zeros_like(x_)
            tau = 0
            alpha = xn
        else:
            v = v / np.linalg.norm(v)
            tau = 2
        w = tau * (A @ v.conj())
        A = A + np.outer(v, w) - np.outer(w, v)
        logpf += np.log(np.abs(1 - tau)) + (np.log(np.abs(-alpha)) if i % 2 == 0 else 0.0)
        sign_val *= ((1 - tau) / np.abs(1 - tau)) * ((-alpha / np.abs(-alpha)) if i % 2 == 0 else 1.0)
    logpf += np.log(np.abs(A[m - 2, m - 1]))
    sign_val *= A[m - 2, m - 1] / np.abs(A[m - 2, m - 1])
    return sign_val, logpf


def _gen_v(zz, PX):
    sgn = np.sign(zz).astype(np.float64).copy()
    sgn[-1] = -PX * sgn[-1]
    norm = 1 / np.sqrt(2.0)
    v = np.zeros((N, n), dtype=np.complex128)
    for k in range(n):
        v[2 * k + 1, k] = -1j * sgn[k] * norm
        v[(2 * k + 2) % N, k] = norm
    return v


def _gf2(L, R):
    M = L.conj().T @ R
    X = np.linalg.solve(M, L.conj().T)
    return np.eye(N) - 2 * (R @ X)


def _logeta_g_expH(H):
    Hh = 1j * (H - H.T) / 2
    e, v = np.linalg.eigh(Hh)
    green = np.real(v @ np.diag(1j * np.tan(e / 2)) @ v.conj().T)
    e_pos = e[: N // 2]
    logeta = np.sum(np.log(np.cos(e_pos / 2).astype(np.complex128)))
    expH = v @ np.diag(np.exp(-1j * e)) @ v.conj().T
    return logeta, green, expH


def _plus_state():
    st = np.zeros((N, n), dtype=np.complex128)
    for k in range(n):
        st[2 * k, k] = -1j / np.sqrt(2)
        st[2 * k + 1, k] = 1 / np.sqrt(2)
    return st


def _minus_state():
    st = np.zeros((N, n), dtype=np.complex128)
    for k in range(n):
        st[2 * k, k] = (1j if k == n - 1 else -1j) / np.sqrt(2)
        st[2 * k + 1, k] = 1 / np.sqrt(2)
    return st


def _log_eta_prop(G1, G2, l1, l2):
    A = (G1 - G1.T) * 0.5
    D = (G2 - G2.T) * 0.5
    pfmat = np.block([[A, -np.eye(N)], [np.eye(N), D]])
    sign_pref = (-1) ** (N // 2)
    s, l = _slog_pf(pfmat)
    return l1 + l2 + np.log(sign_pref * s) + l


def _sector_setup(R, Ghz, logeta_Ghz, PX):
    A = (Ghz - Ghz.T) * 0.5
    Ea = np.zeros((N, n))
    Eb = np.zeros((N, n))
    for k in range(n):
        Ea[2 * k + 1, k] = 1 / np.sqrt(2.0)
        Eb[(2 * k + 2) % N, k] = 1 / np.sqrt(2.0)
    m1 = Ea.T @ R
    m0 = Eb.T @ R
    F11 = R.T @ A @ R
    F11inv = np.linalg.inv(F11)
    P1 = m1.T + R.T @ A @ Ea
    P0 = m0.T + R.T @ A @ Eb
    q11 = Ea.T @ A @ Ea
    q12 = Ea.T @ A @ Eb
    q21 = Eb.T @ A @ Ea
    q22 = Eb.T @ A @ Eb
    Z11 = q11 + P1.T @ F11inv @ P1
    Z10 = q12 + P1.T @ F11inv @ P0
    Z01 = q21 + P0.T @ F11inv @ P1
    Z00 = q22 + P0.T @ F11inv @ P0
    Ainv = np.linalg.inv(A)
    sA, lA = _slog_pf(A)
    sAi, lAi = _slog_pf(Ainv)
    sF, lF = _slog_pf(F11)
    # det(M) is the same for every valid sigma (parity constrained); use x=ones
    xr = np.ones(n)
    zzr = xr * np.roll(xr, -1)
    sig = np.sign(zzr)
    sig[-1] *= -PX
    Ls = Ea * (-1j * sig)[None, :] + Eb
    detM = np.linalg.det(Ls.conj().T @ R)
    logC = (logeta_Ghz + np.log(sA) + lA + np.log(sAi) + lAi
            + np.log(sF) + lF - np.log(detM))
    return dict(Z11=Z11, Z10=Z10, Z01=Z01, Z00=Z00, logC=logC)


_setup_cache = {}
_nc_cache = None


def _shared_setup(s0, H1, H2):
    key = (s0.tobytes(), H1.tobytes(), H2.tobytes())
    if key in _setup_cache:
        return _setup_cache[key]
    ps, ms = _plus_state(), _minus_state()
    zz0 = s0 * np.roll(s0, -1)
    v_plus = _gen_v(zz0, 1)
    v_minus = _gen_v(zz0, -1)
    Gz_plus = _gf2(v_plus, v_plus)
    Gz_minus = _gf2(v_minus, v_minus)
    le_p, G_p, expH_p = _logeta_g_expH(H1)
    le_m, G_m, expH_m = _logeta_g_expH(H2)
    Ghz_plus = _gf2(v_plus, expH_p @ v_plus)
    Ghz_minus = _gf2(v_minus, expH_m @ v_minus)
    logeta_Ghz_plus = _log_eta_prop(G_p, Gz_plus, le_p, 0.0)
    logeta_Ghz_minus = _log_eta_prop(G_m, Gz_minus, le_m, 0.0)
    sp = _sector_setup(ps, Ghz_plus, logeta_Ghz_plus, 1)
    sm = _sector_setup(ms, Ghz_minus, logeta_Ghz_minus, -1)
    K_p = np.exp(sp['logC'] - 16 * np.log(CSCALE) - SHIFT)
    K_m = np.exp(sm['logC'] - 16 * np.log(CSCALE) - SHIFT)

    bones = np.zeros((P, P), np.float32)
    for p1 in range(P):
        bones[p1, (p1 // 32) * 32:(p1 // 32) * 32 + 32] = 1.0
    rmask = np.zeros((P, 31), np.float32)
    for j in range(31):
        rmask[:, j] = (np.arange(P) % 32 == j)

    res = dict(sp=sp, sm=sm, bones=bones, rmask=rmask, K_p=K_p, K_m=K_m)
    _setup_cache[key] = res
    return res


# ----------------------------------------------------------------------------
# device program
# ----------------------------------------------------------------------------

def _build_nc(nsteps=15, dump=False):
    global _nc_cache
    if _nc_cache is not None and nsteps == 15 and not dump:
        return _nc_cache
    nc = bacc.Bacc()
    scat_d = nc.dram_tensor("scat0", [P, 256], f32, kind="ExternalInput")
    bones_d = nc.dram_tensor("bones", [P, P], f32, kind="ExternalInput")
    rmask_d = nc.dram_tensor("rmask", [P, 31], f32, kind="ExternalInput")
    outp_d = nc.dram_tensor("outp", [P, 8 * nsteps], f32, kind="ExternalOutput")
    # pivots pass through SBUF (DMA cannot read PSUM)
    outt_d = nc.dram_tensor("outt", [P, 16], f32, kind="ExternalOutput")

    with tile.TileContext(nc) as tc:
        with tc.tile_pool(name="const", bufs=1) as cpool, \
             tc.tile_pool(name="state", bufs=1) as spool, \
             tc.tile_pool(name="temps", bufs=2) as tpool, \
             tc.tile_pool(name="psum", bufs=2, space="PSUM") as ppool:

            # dummy scalar op with no DMA deps: forces ACT_TABLE_LOAD to
            # overlap the input DMAs instead of serializing after them
            warm = cpool.tile([P, 1], f32, tag="warm")
            nc.gpsimd.memset(warm[:], 0.0)
            nc.scalar.copy(warm[:], warm[:])

            Scat = spool.tile([P, 2, 4, 32], f32, tag="Scat")
            nc.sync.dma_start(Scat[:].rearrange("p c g w -> p (c g w)"), scat_d[:])
            bones = cpool.tile([P, P], f32, tag="bones")
            nc.sync.dma_start(bones[:], bones_d[:])
            rmask = cpool.tile([P, 31], f32, tag="rmask")
            nc.sync.dma_start(rmask[:], rmask_d[:])

            # selectors for EVEN rows only (row k = 2s): selt[:, s, :]
            selt = cpool.tile([P, nsteps, P], f32, tag="selt")

            def build_sel(si):
                nc.scalar.mul(selt[:, si, :], bones[:], rmask[:, 2 * si:2 * si + 1])

            build_sel(0)
            if nsteps > 1:
                build_sel(1)

            Scat2 = Scat[:].rearrange("p c g w -> p (c g w)")
            pivstore = spool.tile([P, nsteps, 2, 4], f32, tag="piv")

            for s in range(nsteps):
                k = 2 * s
                w2 = 30 - k

                # broadcast row k of every matrix across its partition block
                apl = ppool.tile([P, 2, 4, 32], f32, tag="apl")
                nc.tensor.matmul(
                    apl[:].rearrange("p c g w -> p (c g w)"),
                    selt[:, s, :], Scat2, start=True, stop=True)

                # pivot pi = S[k, k+1] = apl[:, :, :, k+1] -> SBUF (scalar)
                nc.scalar.copy(pivstore[:, s], apl[:, :, :, k + 1])

                if s + 2 < nsteps:
                    build_sel(s + 2)

                # |pi|^2 on scalar (PSUM-capable), den on gpsimd, rec on vector
                sq = tpool.tile([P, 2, 4], f32, tag="sq")
                nc.scalar.square(sq[:], apl[:, :, :, k + 1])
                den = tpool.tile([P, 4], f32, tag="den")
                nc.gpsimd.tensor_add(den[:], sq[:, 0], sq[:, 1])
                rec = tpool.tile([P, 4], f32, tag="rec")
                nc.vector.reciprocal(rec[:], den[:])

                # u2 = conj(pi) * S[:,k+1] * rec  (column read, no broadcast)
                #   u2r = (c2r*pvr + c2i*pvi)*rec ; u2i = (c2i*pvr - c2r*pvi)*rec
                cpair = Scat[:, :, :, k + 1]                      # [P, 2ch, 4g]
                pvr_b = apl[:, 0, None, :, k + 1].broadcast_to([P, 2, 4])
                pvi_b = apl[:, 1, None, :, k + 1].broadcast_to([P, 2, 4])
                P13 = tpool.tile([P, 2, 4], f32, tag="P13")
                nc.vector.tensor_mul(P13[:], cpair, pvr_b)
                P24 = tpool.tile([P, 2, 4], f32, tag="P24")
                nc.vector.tensor_mul(P24[:], cpair, pvi_b)
                N2 = tpool.tile([P, 2, 4], f32, tag="N2")
                nc.vector.tensor_add(N2[:, 0], P13[:, 0], P24[:, 1])
                nc.vector.tensor_sub(N2[:, 1], P13[:, 1], P24[:, 0])
                # UU: [u2r | u2i | -u2r] each [P, 4]
                UU = tpool.tile([P, 3, 4], f32, tag="UU")
                nc.vector.tensor_mul(UU[:, 0:2], N2[:],
                                     rec[:, None, :].broadcast_to([P, 2, 4]))
                nc.vector.tensor_scalar(out=UU[:, 2], in0=UU[:, 0], scalar1=-1.0,
                                        scalar2=None, op0=AOT.mult)

                # T = u2 (x) (-row_k):  T = mB - mA with
                #   mA = AR*(u2r,u2i), mB = AI*(u2i,-u2r)
                AR_b = apl[:, 0, None, :, :].broadcast_to([P, 2, 4, 32])
                AI_b = apl[:, 1, None, :, :].broadcast_to([P, 2, 4, 32])
                vA = UU[:, 0:2, :, None].broadcast_to([P, 2, 4, 32])
                vB = UU[:, 1:3, :, None].broadcast_to([P, 2, 4, 32])
                mA = tpool.tile([P, 2, 4, 32], f32, tag="mA")
                nc.vector.tensor_mul(mA[:], AR_b, vA)
                mB = tpool.tile([P, 2, 4, 32], f32, tag="mB")
                nc.vector.tensor_mul(mB[:], AI_b, vB)
                T = tpool.tile([P, 2, 4, 32], f32, tag="T")
                nc.vector.tensor_sub(T[:], mB[:], mA[:])

                # S += T - T^T on the trailing columns
                TT = tpool.tile([P, 2, 4, 32], f32, tag="TT")
                nc.vector.transpose(TT[:].rearrange("p c g w -> p (c g w)"),
                                    T[:].rearrange("p c g w -> p (c g w)"))
                Str = Scat[:, :, :, k + 2:]
                nc.vector.scalar_tensor_tensor(
                    out=Str, in0=T[:, :, :, k + 2:], scalar=1.0, in1=Str,
                    op0=AOT.mult, op1=AOT.add)
                nc.vector.scalar_tensor_tensor(
                    out=Str, in0=TT[:, :, :, k + 2:], scalar=-1.0, in1=Str,
                    op0=AOT.mult, op1=AOT.add)

            if dump:
                dump_d = nc.dram_tensor("dump_s", [P, 256], f32, kind="ExternalOutput")
                nc.sync.dma_start(dump_d[:], Scat2)

            nc.sync.dma_start(outp_d[:],
                              pivstore[:].rearrange("p s c g -> p (s c g)"))
            # final pivot S[30, 31] read host-side from the last two columns
            nc.sync.dma_start(outt_d[:].rearrange("p (c g w) -> p c g w",
                                                  c=2, g=4),
                              Scat[:, :, :, 30:])

    nc.compile()
    if nsteps == 15 and not dump:
        _nc_cache = nc
    return nc


# ----------------------------------------------------------------------------
# entry point
# ----------------------------------------------------------------------------

def kernel(x, s0, H1, H2):
    global LAST_RESULTS
    x64 = np.asarray(x, dtype=np.float64)
    s064 = np.asarray(s0, dtype=np.float64)
    H164 = np.asarray(H1, dtype=np.float64)
    H264 = np.asarray(H2, dtype=np.float64)
    B = x64.shape[0]
    assert B == 64 and x64.shape[1] == n

    st = _shared_setup(s064, H164, H264)
    nc = _build_nc()

    zz = x64 * np.roll(x64, -1, axis=1)
    sgn = np.sign(zz)

    zm = {}
    for plus, sd in ((True, st['sp']), (False, st['sm'])):
        zm[plus] = tuple((sd[kk] * CSCALE).astype(np.complex128)
                         for kk in ('Z11', 'Z10', 'Z01', 'Z00'))

    in_maps = []
    for c in range(NCORES):
        scat0 = np.zeros((P, 2, 4, 32), np.float32)
        for mi in range(4):
            for gi in range(4):
                samp = c * 8 + mi * 2 + gi // 2
                plus = (gi % 2 == 0)
                Z11, Z10, Z01, Z00 = zm[plus]
                sig = sgn[samp].copy()
                sig[-1] *= -1.0 if plus else 1.0
                S = (Z00 - np.outer(sig, sig) * Z11
                     + 1j * sig[:, None] * Z10 + 1j * sig[None, :] * Z01)
                scat0[mi * 32:mi * 32 + 32, 0, gi, :] = S.real
                scat0[mi * 32:mi * 32 + 32, 1, gi, :] = S.imag
        in_maps.append(dict(scat0=scat0.reshape(P, 256),
                            bones=st['bones'], rmask=st['rmask']))

    trace = bool(int(os.environ.get("PFK_TRACE", "0")))
    res = run_bass_kernel_spmd(nc, in_maps, core_ids=list(range(NCORES)),
                               trace=trace)
    LAST_RESULTS = res

    out = np.zeros(B, dtype=np.complex128)
    xs_last = x64[:, -1] * s064[-1]
    for c in range(NCORES):
        op = np.asarray(res.results[c]["outp"], dtype=np.float64)
        ot = np.asarray(res.results[c]["outt"], dtype=np.float64)
        for mi in range(4):
            pm = op[32 * mi].reshape(15, 2, 4)         # [s, ch, g]
            pc = pm[:, 0, :] + 1j * pm[:, 1, :]        # [15, 4g]
            tl = ot[32 * mi + 30].reshape(2, 4, 2)     # [ch, g, col(30,31)]
            fin = tl[0, :, 1] + 1j * tl[1, :, 1]       # [4g]
            pf = np.prod(pc, axis=0) * fin             # [4g]
            for j in range(2):
                samp = c * 8 + mi * 2 + j
                E_p = st['K_p'] * xs_last[samp] * pf[2 * j]
                E_m = st['K_m'] * pf[2 * j + 1]
                out[samp] = np.log(E_m + E_p) + SHIFT
    return out


# revision 11
# speedup vs baseline: 1.6372x; 1.1898x over previous
"""Trainium kernel for nn_EpsilonState: batched log-amplitude of Gaussian-state
overlaps.

Math: each sample reduces to a pair of 32x32 complex skew Pfaffians S built
elementwise from four shared 32x32 matrices (host-side), sign-modulated by the
sample's sigma vector:

    S = Z00 - (sig sig^T).Z11 + i (sig 1^T).Z10 + i (1 sig^T).Z01   (x CSCALE)
    Pf(S) = prod_s pivots of Parlett-Reid elimination (no pivoting)
    out_b = log(K_m Pf_m + x_b[31] K_p Pf_p) + SHIFT                 (host)

Device: 8 cores x 16 matrices (8 samples x 2 sectors); each core holds its 16
matrices as [128p, 2ch, 4g, 32c] fp32 (4 partition blocks x 4 column groups)
and runs 15 Parlett-Reid steps. Row broadcasts are fp32r selector matmuls on
the tensor engine (exact: weights are 0/1); column vectors are read directly
from S (skew symmetry) so only the pivot needs the broadcast. Pivots are
stored per step and the complex product is taken on the host in float64.
Rank-2 updates are split vector (a-rows) / gpsimd (b-rows, via a scalar-engine
PSUM->SBUF copy since gpsimd has no PSUM port).
"""
import os
import numpy as np

import concourse.bass as bass
from concourse import bacc
import concourse.mybir as mybir
import concourse.tile as tile
from concourse.bass_utils import run_bass_kernel_spmd

f32 = mybir.dt.float32
f32r = mybir.dt.float32r
P = 128
n = 32
N = 64
NCORES = 8
CSCALE = 64.0
SHIFT = -51.0
AOT = mybir.AluOpType

LAST_RESULTS = None  # stash of BassKernelResults for test harness introspection


# ----------------------------------------------------------------------------
# host-side shared setup (float64 numpy; depends only on s0, H1, H2)
# ----------------------------------------------------------------------------

def _slog_pf(A):
    A = A.copy()
    m = A.shape[0]
    sign_val = 1.0 + 0j
    logpf = 0.0
    for i in range(m - 2):
        x_ = A[:, i].copy()
        nidx = i + 1
        ar = np.arange(m)
        xn = x_[nidx]
        x_[ar <= nidx] = 0
        sigma = np.vdot(x_, x_)
        norm_x = np.sqrt(xn.conj() * xn + sigma)
        phase = 1.0 if xn == 0 else xn / np.abs(xn)
        vn = xn + phase * norm_x
        alpha = -phase * norm_x
        v = x_.copy()
        v[nidx] = vn
        if sigma == 0:
            v = np.zeros_like(x_)
            tau = 0
            alpha = xn
        else:
            v = v / np.linalg.norm(v)
            tau = 2
        w = tau * (A @ v.conj())
        A = A + np.outer(v, w) - np.outer(w, v)
        logpf += np.log(np.abs(1 - tau)) + (np.log(np.abs(-alpha)) if i % 2 == 0 else 0.0)
        sign_val *= ((1 - tau) / np.abs(1 - tau)) * ((-alpha / np.abs(-alpha)) if i % 2 == 0 else 1.0)
    logpf += np.log(np.abs(A[m - 2, m - 1]))
    sign_val *= A[m - 2, m - 1] / np.abs(A[m - 2, m - 1])
    return sign_val, logpf


def _gen_v(zz, PX):
    sgn = np.sign(zz).astype(np.float64).copy()
    sgn[-1] = -PX * sgn[-1]
    norm = 1 / np.sqrt(2.0)
    v = np.zeros((N, n), dtype=np.complex128)
    for k in range(n):
        v[2 * k + 1, k] = -1j * sgn[k] * norm
        v[(2 * k + 2) % N, k] = norm
    return v


def _gf2(L, R):
    M = L.conj().T @ R
    X = np.linalg.solve(M, L.conj().T)
    return np.eye(N) - 2 * (R @ X)


def _logeta_g_expH(H):
    Hh = 1j * (H - H.T) / 2
    e, v = np.linalg.eigh(Hh)
    green = np.real(v @ np.diag(1j * np.tan(e / 2)) @ v.conj().T)
    e_pos = e[: N // 2]
    logeta = np.sum(np.log(np.cos(e_pos / 2).astype(np.complex128)))
    expH = v @ np.diag(np.exp(-1j * e)) @ v.conj().T
    return logeta, green, expH


def _plus_state():
    st = np.zeros((N, n), dtype=np.complex128)
    for k in range(n):
        st[2 * k, k] = -1j / np.sqrt(2)
        st[2 * k + 1, k] = 1 / np.sqrt(2)
    return st


def _minus_state():
    st = np.zeros((N, n), dtype=np.complex128)
    for k in range(n):
        st[2 * k, k] = (1j if k == n - 1 else -1j) / np.sqrt(2)
        st[2 * k + 1, k] = 1 / np.sqrt(2)
    return st


def _log_eta_prop(G1, G2, l1, l2):
    A = (G1 - G1.T) * 0.5
    D = (G2 - G2.T) * 0.5
    pfmat = np.block([[A, -np.eye(N)], [np.eye(N), D]])
    sign_pref = (-1) ** (N // 2)
    s, l = _slog_pf(pfmat)
    return l1 + l2 + np.log(sign_pref * s) + l


def _sector_setup(R, Ghz, logeta_Ghz, PX):
    A = (Ghz - Ghz.T) * 0.5
    Ea = np.zeros((N, n))
    Eb = np.zeros((N, n))
    for k in range(n):
        Ea[2 * k + 1, k] = 1 / np.sqrt(2.0)
        Eb[(2 * k + 2) % N, k] = 1 / np.sqrt(2.0)
    m1 = Ea.T @ R
    m0 = Eb.T @ R
    F11 = R.T @ A @ R
    F11inv = np.linalg.inv(F11)
    P1 = m1.T + R.T @ A @ Ea
    P0 = m0.T + R.T @ A @ Eb
    q11 = Ea.T @ A @ Ea
    q12 = Ea.T @ A @ Eb
    q21 = Eb.T @ A @ Ea
    q22 = Eb.T @ A @ Eb
    Z11 = q11 + P1.T @ F11inv @ P1
    Z10 = q12 + P1.T @ F11inv @ P0
    Z01 = q21 + P0.T @ F11inv @ P1
    Z00 = q22 + P0.T @ F11inv @ P0
    Ainv = np.linalg.inv(A)
    sA, lA = _slog_pf(A)
    sAi, lAi = _slog_pf(Ainv)
    sF, lF = _slog_pf(F11)
    # det(M) is the same for every valid sigma (parity constrained); use x=ones
    xr = np.ones(n)
    zzr = xr * np.roll(xr, -1)
    sig = np.sign(zzr)
    sig[-1] *= -PX
    Ls = Ea * (-1j * sig)[None, :] + Eb
    detM = np.linalg.det(Ls.conj().T @ R)
    logC = (logeta_Ghz + np.log(sA) + lA + np.log(sAi) + lAi
            + np.log(sF) + lF - np.log(detM))
    return dict(Z11=Z11, Z10=Z10, Z01=Z01, Z00=Z00, logC=logC)


_setup_cache = {}
_nc_cache = None


def _shared_setup(s0, H1, H2):
    key = (s0.tobytes(), H1.tobytes(), H2.tobytes())
    if key in _setup_cache:
        return _setup_cache[key]
    ps, ms = _plus_state(), _minus_state()
    zz0 = s0 * np.roll(s0, -1)
    v_plus = _gen_v(zz0, 1)
    v_minus = _gen_v(zz0, -1)
    Gz_plus = _gf2(v_plus, v_plus)
    Gz_minus = _gf2(v_minus, v_minus)
    le_p, G_p, expH_p = _logeta_g_expH(H1)
    le_m, G_m, expH_m = _logeta_g_expH(H2)
    Ghz_plus = _gf2(v_plus, expH_p @ v_plus)
    Ghz_minus = _gf2(v_minus, expH_m @ v_minus)
    logeta_Ghz_plus = _log_eta_prop(G_p, Gz_plus, le_p, 0.0)
    logeta_Ghz_minus = _log_eta_prop(G_m, Gz_minus, le_m, 0.0)
    sp = _sector_setup(ps, Ghz_plus, logeta_Ghz_plus, 1)
    sm = _sector_setup(ms, Ghz_minus, logeta_Ghz_minus, -1)
    K_p = np.exp(sp['logC'] - 16 * np.log(CSCALE) - SHIFT)
    K_m = np.exp(sm['logC'] - 16 * np.log(CSCALE) - SHIFT)

    import ml_dtypes
    bones = np.zeros((P, P), np.float32)
    for p1 in range(P):
        bones[p1, (p1 // 32) * 32:(p1 // 32) * 32 + 32] = 1.0
    bones = bones.astype(ml_dtypes.bfloat16)
    rmask = np.zeros((P, 31), np.float32)
    for j in range(31):
        rmask[:, j] = (np.arange(P) % 32 == j)

    res = dict(sp=sp, sm=sm, bones=bones, rmask=rmask, K_p=K_p, K_m=K_m)
    _setup_cache[key] = res
    return res


# ----------------------------------------------------------------------------
# device program
# ----------------------------------------------------------------------------
# S layout is c-major: Scat [P, 32c, 2ch, 4g] so that the matmul rhs for
# columns >= k+1 is a contiguous 2D tail slice (the ISA only allows 2D
# matmul operands). Stale values in columns < k+2 of T / rows < k+2 of S
# are never read by later steps (only the trailing block matters).

def _build_nc(nsteps=15, dump=False):
    global _nc_cache
    if _nc_cache is not None and nsteps == 15 and not dump:
        return _nc_cache
    nc = bacc.Bacc()
    scat_d = nc.dram_tensor("scat0", [P, 256], f32, kind="ExternalInput")
    bones_d = nc.dram_tensor("bones", [P, P], mybir.dt.bfloat16, kind="ExternalInput")
    rmask_d = nc.dram_tensor("rmask", [P, 31], f32, kind="ExternalInput")
    outp_d = nc.dram_tensor("outp", [P, 8 * nsteps], f32, kind="ExternalOutput")
    outt_d = nc.dram_tensor("outt", [P, 16], f32, kind="ExternalOutput")

    with tile.TileContext(nc) as tc:
        with tc.tile_pool(name="const", bufs=1) as cpool, \
             tc.tile_pool(name="state", bufs=1) as spool, \
             tc.tile_pool(name="temps", bufs=2) as tpool, \
             tc.tile_pool(name="psum", bufs=2, space="PSUM") as ppool:

            # dummy scalar op with no DMA deps: ACT_TABLE_LOAD overlaps DMAs
            warm = cpool.tile([P, 1], f32, tag="warm")
            nc.gpsimd.memset(warm[:], 0.0)
            nc.scalar.copy(warm[:], warm[:])

            Scat = spool.tile([P, 32, 2, 4], f32, tag="Scat")
            nc.sync.dma_start(Scat[:].rearrange("p c e g -> p (c e g)"), scat_d[:])
            bones = cpool.tile([P, P], mybir.dt.bfloat16, tag="bones")
            nc.sync.dma_start(bones[:], bones_d[:])
            rmask = cpool.tile([P, 31], f32, tag="rmask")
            nc.sync.dma_start(rmask[:], rmask_d[:])

            selt = cpool.tile([P, nsteps, P], f32, tag="selt")

            def build_sel(si):
                nc.scalar.mul(selt[:, si, :], bones[:], rmask[:, 2 * si:2 * si + 1])

            build_sel(0)
            if nsteps > 1:
                build_sel(1)

            Scat2 = Scat[:].rearrange("p c e g -> p (c e g)")
            pivstore = spool.tile([P, nsteps, 2, 4], f32, tag="piv")

            for s in range(nsteps):
                k = 2 * s
                w1 = 31 - k   # broadcast columns k+1..31; apl c-index j = col k+1+j
                w2 = 30 - k

                apl = ppool.tile([P, 32, 2, 4], f32, tag="apl")
                nc.tensor.matmul(
                    apl[:].rearrange("p c e g -> p (c e g)")[:, :8 * w1],
                    selt[:, s, :], Scat2[:, 8 * (k + 1):], start=True, stop=True)

                # |pi|^2 on scalar (PSUM-capable), then pivot copy for output
                sq = tpool.tile([P, 2, 4], f32, tag="sq")
                nc.scalar.square(sq[:], apl[:, 0])
                nc.scalar.copy(pivstore[:, s], apl[:, 0])
                if s + 2 < nsteps:
                    build_sel(s + 2)

                den = tpool.tile([P, 4], f32, tag="den")
                nc.vector.tensor_add(den[:], sq[:, 0], sq[:, 1])
                rec = tpool.tile([P, 4], f32, tag="rec")
                nc.vector.reciprocal(rec[:], den[:])

                # u2 = conj(pi)*S[:,k+1]*rec; numerators first, rec folded in
                cpair = Scat[:, k + 1]                            # [P, 2ch, 4g]
                pvr_b = apl[:, 0, 0, None, :].broadcast_to([P, 2, 4])
                pvi_b = apl[:, 0, 1, None, :].broadcast_to([P, 2, 4])
                P13 = tpool.tile([P, 2, 4], f32, tag="P13")
                nc.vector.tensor_mul(P13[:], cpair, pvr_b)
                P24 = tpool.tile([P, 2, 4], f32, tag="P24")
                nc.vector.tensor_mul(P24[:], cpair, pvi_b)
                N2 = tpool.tile([P, 2, 4], f32, tag="N2")
                nc.vector.tensor_add(N2[:, 0], P13[:, 0], P24[:, 1])
                nc.vector.tensor_sub(N2[:, 1], P13[:, 1], P24[:, 0])
                UU = tpool.tile([P, 3, 4], f32, tag="UU")
                nc.vector.tensor_mul(UU[:, 0:2], N2[:],
                                     rec[:, None, :].broadcast_to([P, 2, 4]))
                nc.vector.tensor_scalar(out=UU[:, 2], in0=UU[:, 0], scalar1=-1.0,
                                        scalar2=None, op0=AOT.mult)

                # T = u2 (x) (-row_k) on columns >= k+2 (T tile is c-major)
                AR_b = apl[:, 1:w1, 0, None, :].broadcast_to([P, w2, 2, 4])
                AI_b = apl[:, 1:w1, 1, None, :].broadcast_to([P, w2, 2, 4])
                vA = UU[:, None, 0:2, :].broadcast_to([P, w2, 2, 4])
                vB = UU[:, None, 1:3, :].broadcast_to([P, w2, 2, 4])
                mA = tpool.tile([P, 32, 2, 4], f32, tag="mA")
                nc.vector.tensor_mul(mA[:, k + 2:], AR_b, vA)
                mB = tpool.tile([P, 32, 2, 4], f32, tag="mB")
                nc.vector.tensor_mul(mB[:, k + 2:], AI_b, vB)
                T = spool.tile([P, 32, 2, 4], f32, tag="T")
                nc.vector.tensor_sub(T[:, k + 2:], mB[:, k + 2:], mA[:, k + 2:])

                # S += T - T^T on trailing columns (stream order (e,g,c))
                TT = spool.tile([P, 32, 2, 4], f32, tag="TT")
                nc.vector.transpose(TT[:].transpose([0, 2, 3, 1]),
                                    T[:].transpose([0, 2, 3, 1]))
                Str = Scat[:, k + 2:]
                nc.vector.scalar_tensor_tensor(
                    out=Str, in0=T[:, k + 2:], scalar=1.0, in1=Str,
                    op0=AOT.mult, op1=AOT.add)
                nc.vector.scalar_tensor_tensor(
                    out=Str, in0=TT[:, k + 2:], scalar=-1.0, in1=Str,
                    op0=AOT.mult, op1=AOT.add)

            if dump:
                dump_d = nc.dram_tensor("dump_s", [P, 256], f32, kind="ExternalOutput")
                nc.sync.dma_start(dump_d[:], Scat2)

            nc.sync.dma_start(outp_d[:],
                              pivstore[:].rearrange("p s c g -> p (s c g)"))
            # final pivot S[30, 31] read host-side from the last two columns
            nc.sync.dma_start(outt_d[:].rearrange("p (c e g) -> p c e g",
                                                  c=2, e=2),
                              Scat[:, 30:])

    nc.compile()
    if nsteps == 15 and not dump:
        _nc_cache = nc
    return nc


# ----------------------------------------------------------------------------
# entry point
# ----------------------------------------------------------------------------

def kernel(x, s0, H1, H2):
    global LAST_RESULTS
    x64 = np.asarray(x, dtype=np.float64)
    s064 = np.asarray(s0, dtype=np.float64)
    H164 = np.asarray(H1, dtype=np.float64)
    H264 = np.asarray(H2, dtype=np.float64)
    B = x64.shape[0]
    assert B == 64 and x64.shape[1] == n

    st = _shared_setup(s064, H164, H264)
    nc = _build_nc()

    zz = x64 * np.roll(x64, -1, axis=1)
    sgn = np.sign(zz)

    zm = {}
    for plus, sd in ((True, st['sp']), (False, st['sm'])):
        zm[plus] = tuple((sd[kk] * CSCALE).astype(np.complex128)
                         for kk in ('Z11', 'Z10', 'Z01', 'Z00'))

    in_maps = []
    for c in range(NCORES):
        scat0 = np.zeros((P, 32, 2, 4), np.float32)   # c-major layout
        for mi in range(4):
            for gi in range(4):
                samp = c * 8 + mi * 2 + gi // 2
                plus = (gi % 2 == 0)
                Z11, Z10, Z01, Z00 = zm[plus]
                sig = sgn[samp].copy()
                sig[-1] *= -1.0 if plus else 1.0
                S = (Z00 - np.outer(sig, sig) * Z11
                     + 1j * sig[:, None] * Z10 + 1j * sig[None, :] * Z01)
                scat0[mi * 32:mi * 32 + 32, :, 0, gi] = S.real
                scat0[mi * 32:mi * 32 + 32, :, 1, gi] = S.imag
        in_maps.append(dict(scat0=scat0.reshape(P, 256),
                            bones=st['bones'], rmask=st['rmask']))

    trace = bool(int(os.environ.get("PFK_TRACE", "0")))
    res = run_bass_kernel_spmd(nc, in_maps, core_ids=list(range(NCORES)),
                               trace=trace)
    LAST_RESULTS = res

    out = np.zeros(B, dtype=np.complex128)
    xs_last = x64[:, -1] * s064[-1]
    for c in range(NCORES):
        op = np.asarray(res.results[c]["outp"], dtype=np.float64)
        ot = np.asarray(res.results[c]["outt"], dtype=np.float64)
        for mi in range(4):
            pm = op[32 * mi].reshape(15, 2, 4)         # [s, ch, g]
            pc = pm[:, 0, :] + 1j * pm[:, 1, :]        # [15, 4g]
            tl = ot[32 * mi + 30].reshape(2, 2, 4)     # [c(30,31), ch, g]
            fin = tl[1, 0, :] + 1j * tl[1, 1, :]       # [4g]
            pf = np.prod(pc, axis=0) * fin             # [4g]
            for j in range(2):
                samp = c * 8 + mi * 2 + j
                E_p = st['K_p'] * xs_last[samp] * pf[2 * j]
                E_m = st['K_m'] * pf[2 * j + 1]
                out[samp] = np.log(E_m + E_p) + SHIFT
    return out


# revision 12
# speedup vs baseline: 1.7069x; 1.0425x over previous
"""Trainium kernel for nn_EpsilonState: batched log-amplitude of Gaussian-state
overlaps.

Math: each sample reduces to a pair of 32x32 complex skew Pfaffians S built
elementwise from four shared 32x32 matrices (host-side), sign-modulated by the
sample's sigma vector:

    S = Z00 - (sig sig^T).Z11 + i (sig 1^T).Z10 + i (1 sig^T).Z01   (x CSCALE)
    Pf(S) = prod_s pivots of Parlett-Reid elimination (no pivoting)
    out_b = log(K_m Pf_m + x_b[31] K_p Pf_p) + SHIFT                 (host)

Device: 8 cores x 16 matrices (8 samples x 2 sectors); each core holds its 16
matrices as [128p, 2ch, 4g, 32c] fp32 (4 partition blocks x 4 column groups)
and runs 15 Parlett-Reid steps. Row broadcasts are fp32r selector matmuls on
the tensor engine (exact: weights are 0/1); column vectors are read directly
from S (skew symmetry) so only the pivot needs the broadcast. Pivots are
stored per step and the complex product is taken on the host in float64.
Rank-2 updates are split vector (a-rows) / gpsimd (b-rows, via a scalar-engine
PSUM->SBUF copy since gpsimd has no PSUM port).
"""
import os
import numpy as np

import concourse.bass as bass
from concourse import bacc
import concourse.mybir as mybir
import concourse.tile as tile
from concourse.bass_utils import run_bass_kernel_spmd

f32 = mybir.dt.float32
f32r = mybir.dt.float32r
P = 128
n = 32
N = 64
NCORES = 8
CSCALE = 64.0
SHIFT = -51.0
AOT = mybir.AluOpType

LAST_RESULTS = None  # stash of BassKernelResults for test harness introspection


# ----------------------------------------------------------------------------
# host-side shared setup (float64 numpy; depends only on s0, H1, H2)
# ----------------------------------------------------------------------------

def _slog_pf(A):
    A = A.copy()
    m = A.shape[0]
    sign_val = 1.0 + 0j
    logpf = 0.0
    for i in range(m - 2):
        x_ = A[:, i].copy()
        nidx = i + 1
        ar = np.arange(m)
        xn = x_[nidx]
        x_[ar <= nidx] = 0
        sigma = np.vdot(x_, x_)
        norm_x = np.sqrt(xn.conj() * xn + sigma)
        phase = 1.0 if xn == 0 else xn / np.abs(xn)
        vn = xn + phase * norm_x
        alpha = -phase * norm_x
        v = x_.copy()
        v[nidx] = vn
        if sigma == 0:
            v = np.zeros_like(x_)
            tau = 0
            alpha = xn
        else:
            v = v / np.linalg.norm(v)
            tau = 2
        w = tau * (A @ v.conj())
        A = A + np.outer(v, w) - np.outer(w, v)
        logpf += np.log(np.abs(1 - tau)) + (np.log(np.abs(-alpha)) if i % 2 == 0 else 0.0)
        sign_val *= ((1 - tau) / np.abs(1 - tau)) * ((-alpha / np.abs(-alpha)) if i % 2 == 0 else 1.0)
    logpf += np.log(np.abs(A[m - 2, m - 1]))
    sign_val *= A[m - 2, m - 1] / np.abs(A[m - 2, m - 1])
    return sign_val, logpf


def _gen_v(zz, PX):
    sgn = np.sign(zz).astype(np.float64).copy()
    sgn[-1] = -PX * sgn[-1]
    norm = 1 / np.sqrt(2.0)
    v = np.zeros((N, n), dtype=np.complex128)
    for k in range(n):
        v[2 * k + 1, k] = -1j * sgn[k] * norm
        v[(2 * k + 2) % N, k] = norm
    return v


def _gf2(L, R):
    M = L.conj().T @ R
    X = np.linalg.solve(M, L.conj().T)
    return np.eye(N) - 2 * (R @ X)


def _logeta_g_expH(H):
    Hh = 1j * (H - H.T) / 2
    e, v = np.linalg.eigh(Hh)
    green = np.real(v @ np.diag(1j * np.tan(e / 2)) @ v.conj().T)
    e_pos = e[: N // 2]
    logeta = np.sum(np.log(np.cos(e_pos / 2).astype(np.complex128)))
    expH = v @ np.diag(np.exp(-1j * e)) @ v.conj().T
    return logeta, green, expH


def _plus_state():
    st = np.zeros((N, n), dtype=np.complex128)
    for k in range(n):
        st[2 * k, k] = -1j / np.sqrt(2)
        st[2 * k + 1, k] = 1 / np.sqrt(2)
    return st


def _minus_state():
    st = np.zeros((N, n), dtype=np.complex128)
    for k in range(n):
        st[2 * k, k] = (1j if k == n - 1 else -1j) / np.sqrt(2)
        st[2 * k + 1, k] = 1 / np.sqrt(2)
    return st


def _log_eta_prop(G1, G2, l1, l2):
    A = (G1 - G1.T) * 0.5
    D = (G2 - G2.T) * 0.5
    pfmat = np.block([[A, -np.eye(N)], [np.eye(N), D]])
    sign_pref = (-1) ** (N // 2)
    s, l = _slog_pf(pfmat)
    return l1 + l2 + np.log(sign_pref * s) + l


def _sector_setup(R, Ghz, logeta_Ghz, PX):
    A = (Ghz - Ghz.T) * 0.5
    Ea = np.zeros((N, n))
    Eb = np.zeros((N, n))
    for k in range(n):
        Ea[2 * k + 1, k] = 1 / np.sqrt(2.0)
        Eb[(2 * k + 2) % N, k] = 1 / np.sqrt(2.0)
    m1 = Ea.T @ R
    m0 = Eb.T @ R
    F11 = R.T @ A @ R
    F11inv = np.linalg.inv(F11)
    P1 = m1.T + R.T @ A @ Ea
    P0 = m0.T + R.T @ A @ Eb
    q11 = Ea.T @ A @ Ea
    q12 = Ea.T @ A @ Eb
    q21 = Eb.T @ A @ Ea
    q22 = Eb.T @ A @ Eb
    Z11 = q11 + P1.T @ F11inv @ P1
    Z10 = q12 + P1.T @ F11inv @ P0
    Z01 = q21 + P0.T @ F11inv @ P1
    Z00 = q22 + P0.T @ F11inv @ P0
    Ainv = np.linalg.inv(A)
    sA, lA = _slog_pf(A)
    sAi, lAi = _slog_pf(Ainv)
    sF, lF = _slog_pf(F11)
    # det(M) is the same for every valid sigma (parity constrained); use x=ones
    xr = np.ones(n)
    zzr = xr * np.roll(xr, -1)
    sig = np.sign(zzr)
    sig[-1] *= -PX
    Ls = Ea * (-1j * sig)[None, :] + Eb
    detM = np.linalg.det(Ls.conj().T @ R)
    logC = (logeta_Ghz + np.log(sA) + lA + np.log(sAi) + lAi
            + np.log(sF) + lF - np.log(detM))
    return dict(Z11=Z11, Z10=Z10, Z01=Z01, Z00=Z00, logC=logC)


_setup_cache = {}
_nc_cache = None


def _shared_setup(s0, H1, H2):
    key = (s0.tobytes(), H1.tobytes(), H2.tobytes())
    if key in _setup_cache:
        return _setup_cache[key]
    ps, ms = _plus_state(), _minus_state()
    zz0 = s0 * np.roll(s0, -1)
    v_plus = _gen_v(zz0, 1)
    v_minus = _gen_v(zz0, -1)
    Gz_plus = _gf2(v_plus, v_plus)
    Gz_minus = _gf2(v_minus, v_minus)
    le_p, G_p, expH_p = _logeta_g_expH(H1)
    le_m, G_m, expH_m = _logeta_g_expH(H2)
    Ghz_plus = _gf2(v_plus, expH_p @ v_plus)
    Ghz_minus = _gf2(v_minus, expH_m @ v_minus)
    logeta_Ghz_plus = _log_eta_prop(G_p, Gz_plus, le_p, 0.0)
    logeta_Ghz_minus = _log_eta_prop(G_m, Gz_minus, le_m, 0.0)
    sp = _sector_setup(ps, Ghz_plus, logeta_Ghz_plus, 1)
    sm = _sector_setup(ms, Ghz_minus, logeta_Ghz_minus, -1)
    K_p = np.exp(sp['logC'] - 16 * np.log(CSCALE) - SHIFT)
    K_m = np.exp(sm['logC'] - 16 * np.log(CSCALE) - SHIFT)

    import ml_dtypes
    bones = np.zeros((P, P), np.float32)
    for p1 in range(P):
        bones[p1, (p1 // 32) * 32:(p1 // 32) * 32 + 32] = 1.0
    bones = bones.astype(ml_dtypes.bfloat16)
    rmask = np.zeros((P, 31), np.float32)
    for j in range(31):
        rmask[:, j] = (np.arange(P) % 32 == j)

    res = dict(sp=sp, sm=sm, bones=bones, rmask=rmask, K_p=K_p, K_m=K_m)
    _setup_cache[key] = res
    return res


# ----------------------------------------------------------------------------
# device program
# ----------------------------------------------------------------------------
# S layout is c-major: Scat [P, 32c, 2ch, 4g] so that the matmul rhs for
# columns >= k+1 is a contiguous 2D tail slice (the ISA only allows 2D
# matmul operands). Stale values in columns < k+2 of T / rows < k+2 of S
# are never read by later steps (only the trailing block matters).

def _build_nc(nsteps=15, dump=False):
    global _nc_cache
    if _nc_cache is not None and nsteps == 15 and not dump:
        return _nc_cache
    nc = bacc.Bacc()
    scat_d = nc.dram_tensor("scat0", [P, 256], f32, kind="ExternalInput")
    bones_d = nc.dram_tensor("bones", [P, P], mybir.dt.bfloat16, kind="ExternalInput")
    rmask_d = nc.dram_tensor("rmask", [P, 31], f32, kind="ExternalInput")
    outp_d = nc.dram_tensor("outp", [P, 8 * nsteps], f32, kind="ExternalOutput")
    outt_d = nc.dram_tensor("outt", [P, 16], f32, kind="ExternalOutput")

    with tile.TileContext(nc) as tc:
        with tc.tile_pool(name="const", bufs=1) as cpool, \
             tc.tile_pool(name="state", bufs=1) as spool, \
             tc.tile_pool(name="temps", bufs=2) as tpool, \
             tc.tile_pool(name="psum", bufs=2, space="PSUM") as ppool:

            # dummy scalar op with no DMA deps: ACT_TABLE_LOAD overlaps DMAs
            warm = cpool.tile([P, 1], f32, tag="warm")
            nc.gpsimd.memset(warm[:], 0.0)
            nc.scalar.copy(warm[:], warm[:])

            bones = cpool.tile([P, P], mybir.dt.bfloat16, tag="bones")
            nc.sync.dma_start(bones[:], bones_d[:])
            rmask = cpool.tile([P, 31], f32, tag="rmask")
            nc.sync.dma_start(rmask[:], rmask_d[:])
            Scat = spool.tile([P, 32, 2, 4], f32, tag="Scat")
            Scat_f = Scat[:].rearrange("p c e g -> p (c e g)")
            nc.sync.dma_start(Scat_f[:, :128], scat_d[:, :128])
            nc.sync.dma_start(Scat_f[:, 128:], scat_d[:, 128:])

            selt = cpool.tile([P, nsteps, P], f32, tag="selt")

            def build_sel(si):
                nc.scalar.mul(selt[:, si, :], bones[:], rmask[:, 2 * si:2 * si + 1])

            build_sel(0)
            if nsteps > 1:
                build_sel(1)

            Scat2 = Scat[:].rearrange("p c e g -> p (c e g)")
            pivstore = spool.tile([P, nsteps, 2, 4], f32, tag="piv")

            for s in range(nsteps):
                k = 2 * s
                w1 = 31 - k   # broadcast columns k+1..31; apl c-index j = col k+1+j
                w2 = 30 - k

                apl = ppool.tile([P, 32, 2, 4], f32, tag="apl")
                nc.tensor.matmul(
                    apl[:].rearrange("p c e g -> p (c e g)")[:, :8 * w1],
                    selt[:, s, :], Scat2[:, 8 * (k + 1):], start=True, stop=True)

                # numerators first (only dep: matmul); |pi|^2 path runs
                # on scalar+vector in parallel and only gates the late Tscale
                cpair = Scat[:, k + 1]                            # [P, 2ch, 4g]
                pvr_b = apl[:, 0, 0, None, :].broadcast_to([P, 2, 4])
                pvi_b = apl[:, 0, 1, None, :].broadcast_to([P, 2, 4])
                P13 = tpool.tile([P, 2, 4], f32, tag="P13")
                nc.vector.tensor_mul(P13[:], cpair, pvr_b)
                P24 = tpool.tile([P, 2, 4], f32, tag="P24")
                nc.vector.tensor_mul(P24[:], cpair, pvi_b)
                # NN: [N2r | N2i | -N2r] (unscaled u2 numerators)
                NN = tpool.tile([P, 3, 4], f32, tag="NN")
                nc.vector.tensor_add(NN[:, 0], P13[:, 0], P24[:, 1])
                nc.vector.tensor_sub(NN[:, 1], P13[:, 1], P24[:, 0])
                nc.vector.tensor_scalar(out=NN[:, 2], in0=NN[:, 0], scalar1=-1.0,
                                        scalar2=None, op0=AOT.mult)

                sq = tpool.tile([P, 2, 4], f32, tag="sq")
                nc.scalar.square(sq[:], apl[:, 0])
                nc.scalar.copy(pivstore[:, s], apl[:, 0])
                if s + 2 < nsteps:
                    build_sel(s + 2)
                den = tpool.tile([P, 4], f32, tag="den")
                nc.vector.tensor_add(den[:], sq[:, 0], sq[:, 1])
                rec = tpool.tile([P, 4], f32, tag="rec")
                nc.vector.reciprocal(rec[:], den[:])

                # T'' = N2 (x) (-row_k); T tiles are e-major so the stream
                # transpose runs on a contiguous view
                AR_b = apl[:, 1:w1, 0, None, :].transpose([0, 2, 3, 1]) \
                    .broadcast_to([P, 2, 4, w2])
                AI_b = apl[:, 1:w1, 1, None, :].transpose([0, 2, 3, 1]) \
                    .broadcast_to([P, 2, 4, w2])
                vA = NN[:, 0:2, :, None].broadcast_to([P, 2, 4, w2])
                vB = NN[:, 1:3, :, None].broadcast_to([P, 2, 4, w2])
                mA = tpool.tile([P, 2, 4, 32], f32, tag="mA")
                nc.vector.tensor_mul(mA[:, :, :, k + 2:], AR_b, vA)
                mB = tpool.tile([P, 2, 4, 32], f32, tag="mB")
                nc.vector.tensor_mul(mB[:, :, :, k + 2:], AI_b, vB)
                Tpp = tpool.tile([P, 2, 4, 32], f32, tag="Tpp")
                nc.vector.tensor_sub(Tpp[:, :, :, k + 2:], mB[:, :, :, k + 2:],
                                     mA[:, :, :, k + 2:])
                T = spool.tile([P, 2, 4, 32], f32, tag="T")
                nc.vector.tensor_mul(T[:, :, :, k + 2:], Tpp[:, :, :, k + 2:],
                                     rec[:, None, :, None].broadcast_to(
                                         [P, 2, 4, w2]))

                TT = spool.tile([P, 2, 4, 32], f32, tag="TT")
                nc.vector.transpose(TT[:].rearrange("p e g c -> p (e g c)"),
                                    T[:].rearrange("p e g c -> p (e g c)"))
                # S += T - T^T on trailing columns (S is c-major: view T/TT)
                Str = Scat[:, k + 2:]
                Tv = T[:].transpose([0, 3, 1, 2])
                TTv = TT[:].transpose([0, 3, 1, 2])
                nc.vector.scalar_tensor_tensor(
                    out=Str, in0=Tv[:, k + 2:], scalar=1.0, in1=Str,
                    op0=AOT.mult, op1=AOT.add)
                nc.vector.scalar_tensor_tensor(
                    out=Str, in0=TTv[:, k + 2:], scalar=-1.0, in1=Str,
                    op0=AOT.mult, op1=AOT.add)

            if dump:
                dump_d = nc.dram_tensor("dump_s", [P, 256], f32, kind="ExternalOutput")
                nc.sync.dma_start(dump_d[:], Scat2)

            nc.sync.dma_start(outp_d[:],
                              pivstore[:].rearrange("p s c g -> p (s c g)"))
            # final pivot S[30, 31] read host-side from the last two columns
            nc.sync.dma_start(outt_d[:].rearrange("p (c e g) -> p c e g",
                                                  c=2, e=2),
                              Scat[:, 30:])

    nc.compile()
    if nsteps == 15 and not dump:
        _nc_cache = nc
    return nc


# ----------------------------------------------------------------------------
# entry point
# ----------------------------------------------------------------------------

def kernel(x, s0, H1, H2):
    global LAST_RESULTS
    x64 = np.asarray(x, dtype=np.float64)
    s064 = np.asarray(s0, dtype=np.float64)
    H164 = np.asarray(H1, dtype=np.float64)
    H264 = np.asarray(H2, dtype=np.float64)
    B = x64.shape[0]
    assert B == 64 and x64.shape[1] == n

    st = _shared_setup(s064, H164, H264)
    nc = _build_nc()

    zz = x64 * np.roll(x64, -1, axis=1)
    sgn = np.sign(zz)

    zm = {}
    for plus, sd in ((True, st['sp']), (False, st['sm'])):
        zm[plus] = tuple((sd[kk] * CSCALE).astype(np.complex128)
                         for kk in ('Z11', 'Z10', 'Z01', 'Z00'))

    in_maps = []
    for c in range(NCORES):
        scat0 = np.zeros((P, 32, 2, 4), np.float32)   # c-major layout
        for mi in range(4):
            for gi in range(4):
                samp = c * 8 + mi * 2 + gi // 2
                plus = (gi % 2 == 0)
                Z11, Z10, Z01, Z00 = zm[plus]
                sig = sgn[samp].copy()
                sig[-1] *= -1.0 if plus else 1.0
                S = (Z00 - np.outer(sig, sig) * Z11
                     + 1j * sig[:, None] * Z10 + 1j * sig[None, :] * Z01)
                scat0[mi * 32:mi * 32 + 32, :, 0, gi] = S.real
                scat0[mi * 32:mi * 32 + 32, :, 1, gi] = S.imag
        in_maps.append(dict(scat0=scat0.reshape(P, 256),
                            bones=st['bones'], rmask=st['rmask']))

    trace = bool(int(os.environ.get("PFK_TRACE", "0")))
    res = run_bass_kernel_spmd(nc, in_maps, core_ids=list(range(NCORES)),
                               trace=trace)
    LAST_RESULTS = res

    out = np.zeros(B, dtype=np.complex128)
    xs_last = x64[:, -1] * s064[-1]
    for c in range(NCORES):
        op = np.asarray(res.results[c]["outp"], dtype=np.float64)
        ot = np.asarray(res.results[c]["outt"], dtype=np.float64)
        for mi in range(4):
            pm = op[32 * mi].reshape(15, 2, 4)         # [s, ch, g]
            pc = pm[:, 0, :] + 1j * pm[:, 1, :]        # [15, 4g]
            tl = ot[32 * mi + 30].reshape(2, 2, 4)     # [c(30,31), ch, g]
            fin = tl[1, 0, :] + 1j * tl[1, 1, :]       # [4g]
            pf = np.prod(pc, axis=0) * fin             # [4g]
            for j in range(2):
                samp = c * 8 + mi * 2 + j
                E_p = st['K_p'] * xs_last[samp] * pf[2 * j]
                E_m = st['K_m'] * pf[2 * j + 1]
                out[samp] = np.log(E_m + E_p) + SHIFT
    return out


# revision 13
# speedup vs baseline: 1.8173x; 1.0647x over previous
"""Trainium kernel for nn_EpsilonState: batched log-amplitude of Gaussian-state
overlaps.

Math: each sample reduces to a pair of 32x32 complex skew Pfaffians S built
elementwise from four shared 32x32 matrices (host-side), sign-modulated by the
sample's sigma vector:

    S = Z00 - (sig sig^T).Z11 + i (sig 1^T).Z10 + i (1 sig^T).Z01   (x CSCALE)
    Pf(S) = prod_s pivots of Parlett-Reid elimination (no pivoting)
    out_b = log(K_m Pf_m + x_b[31] K_p Pf_p) + SHIFT                 (host)

Device: 8 cores x 16 matrices (8 samples x 2 sectors); each core holds its 16
matrices as [128p, 2ch, 4g, 32c] fp32 (4 partition blocks x 4 column groups)
and runs 15 Parlett-Reid steps. Row broadcasts are fp32r selector matmuls on
the tensor engine (exact: weights are 0/1); column vectors are read directly
from S (skew symmetry) so only the pivot needs the broadcast. Pivots are
stored per step and the complex product is taken on the host in float64.
Rank-2 updates are split vector (a-rows) / gpsimd (b-rows, via a scalar-engine
PSUM->SBUF copy since gpsimd has no PSUM port).
"""
import os
import numpy as np

import concourse.bass as bass
from concourse import bacc
import concourse.mybir as mybir
import concourse.tile as tile
from concourse.bass_utils import run_bass_kernel_spmd

f32 = mybir.dt.float32
f32r = mybir.dt.float32r
P = 128
n = 32
N = 64
NCORES = 8
CSCALE = 64.0
SHIFT = -51.0
AOT = mybir.AluOpType

LAST_RESULTS = None  # stash of BassKernelResults for test harness introspection


# ----------------------------------------------------------------------------
# host-side shared setup (float64 numpy; depends only on s0, H1, H2)
# ----------------------------------------------------------------------------

def _slog_pf(A):
    A = A.copy()
    m = A.shape[0]
    sign_val = 1.0 + 0j
    logpf = 0.0
    for i in range(m - 2):
        x_ = A[:, i].copy()
        nidx = i + 1
        ar = np.arange(m)
        xn = x_[nidx]
        x_[ar <= nidx] = 0
        sigma = np.vdot(x_, x_)
        norm_x = np.sqrt(xn.conj() * xn + sigma)
        phase = 1.0 if xn == 0 else xn / np.abs(xn)
        vn = xn + phase * norm_x
        alpha = -phase * norm_x
        v = x_.copy()
        v[nidx] = vn
        if sigma == 0:
            v = np.zeros_like(x_)
            tau = 0
            alpha = xn
        else:
            v = v / np.linalg.norm(v)
            tau = 2
        w = tau * (A @ v.conj())
        A = A + np.outer(v, w) - np.outer(w, v)
        logpf += np.log(np.abs(1 - tau)) + (np.log(np.abs(-alpha)) if i % 2 == 0 else 0.0)
        sign_val *= ((1 - tau) / np.abs(1 - tau)) * ((-alpha / np.abs(-alpha)) if i % 2 == 0 else 1.0)
    logpf += np.log(np.abs(A[m - 2, m - 1]))
    sign_val *= A[m - 2, m - 1] / np.abs(A[m - 2, m - 1])
    return sign_val, logpf


def _gen_v(zz, PX):
    sgn = np.sign(zz).astype(np.float64).copy()
    sgn[-1] = -PX * sgn[-1]
    norm = 1 / np.sqrt(2.0)
    v = np.zeros((N, n), dtype=np.complex128)
    for k in range(n):
        v[2 * k + 1, k] = -1j * sgn[k] * norm
        v[(2 * k + 2) % N, k] = norm
    return v


def _gf2(L, R):
    M = L.conj().T @ R
    X = np.linalg.solve(M, L.conj().T)
    return np.eye(N) - 2 * (R @ X)


def _logeta_g_expH(H):
    Hh = 1j * (H - H.T) / 2
    e, v = np.linalg.eigh(Hh)
    green = np.real(v @ np.diag(1j * np.tan(e / 2)) @ v.conj().T)
    e_pos = e[: N // 2]
    logeta = np.sum(np.log(np.cos(e_pos / 2).astype(np.complex128)))
    expH = v @ np.diag(np.exp(-1j * e)) @ v.conj().T
    return logeta, green, expH


def _plus_state():
    st = np.zeros((N, n), dtype=np.complex128)
    for k in range(n):
        st[2 * k, k] = -1j / np.sqrt(2)
        st[2 * k + 1, k] = 1 / np.sqrt(2)
    return st


def _minus_state():
    st = np.zeros((N, n), dtype=np.complex128)
    for k in range(n):
        st[2 * k, k] = (1j if k == n - 1 else -1j) / np.sqrt(2)
        st[2 * k + 1, k] = 1 / np.sqrt(2)
    return st


def _log_eta_prop(G1, G2, l1, l2):
    A = (G1 - G1.T) * 0.5
    D = (G2 - G2.T) * 0.5
    pfmat = np.block([[A, -np.eye(N)], [np.eye(N), D]])
    sign_pref = (-1) ** (N // 2)
    s, l = _slog_pf(pfmat)
    return l1 + l2 + np.log(sign_pref * s) + l


def _sector_setup(R, Ghz, logeta_Ghz, PX):
    A = (Ghz - Ghz.T) * 0.5
    Ea = np.zeros((N, n))
    Eb = np.zeros((N, n))
    for k in range(n):
        Ea[2 * k + 1, k] = 1 / np.sqrt(2.0)
        Eb[(2 * k + 2) % N, k] = 1 / np.sqrt(2.0)
    m1 = Ea.T @ R
    m0 = Eb.T @ R
    F11 = R.T @ A @ R
    F11inv = np.linalg.inv(F11)
    P1 = m1.T + R.T @ A @ Ea
    P0 = m0.T + R.T @ A @ Eb
    q11 = Ea.T @ A @ Ea
    q12 = Ea.T @ A @ Eb
    q21 = Eb.T @ A @ Ea
    q22 = Eb.T @ A @ Eb
    Z11 = q11 + P1.T @ F11inv @ P1
    Z10 = q12 + P1.T @ F11inv @ P0
    Z01 = q21 + P0.T @ F11inv @ P1
    Z00 = q22 + P0.T @ F11inv @ P0
    Ainv = np.linalg.inv(A)
    sA, lA = _slog_pf(A)
    sAi, lAi = _slog_pf(Ainv)
    sF, lF = _slog_pf(F11)
    # det(M) is the same for every valid sigma (parity constrained); use x=ones
    xr = np.ones(n)
    zzr = xr * np.roll(xr, -1)
    sig = np.sign(zzr)
    sig[-1] *= -PX
    Ls = Ea * (-1j * sig)[None, :] + Eb
    detM = np.linalg.det(Ls.conj().T @ R)
    logC = (logeta_Ghz + np.log(sA) + lA + np.log(sAi) + lAi
            + np.log(sF) + lF - np.log(detM))
    return dict(Z11=Z11, Z10=Z10, Z01=Z01, Z00=Z00, logC=logC)


_setup_cache = {}
_nc_cache = None


def _shared_setup(s0, H1, H2):
    key = (s0.tobytes(), H1.tobytes(), H2.tobytes())
    if key in _setup_cache:
        return _setup_cache[key]
    ps, ms = _plus_state(), _minus_state()
    zz0 = s0 * np.roll(s0, -1)
    v_plus = _gen_v(zz0, 1)
    v_minus = _gen_v(zz0, -1)
    Gz_plus = _gf2(v_plus, v_plus)
    Gz_minus = _gf2(v_minus, v_minus)
    le_p, G_p, expH_p = _logeta_g_expH(H1)
    le_m, G_m, expH_m = _logeta_g_expH(H2)
    Ghz_plus = _gf2(v_plus, expH_p @ v_plus)
    Ghz_minus = _gf2(v_minus, expH_m @ v_minus)
    logeta_Ghz_plus = _log_eta_prop(G_p, Gz_plus, le_p, 0.0)
    logeta_Ghz_minus = _log_eta_prop(G_m, Gz_minus, le_m, 0.0)
    sp = _sector_setup(ps, Ghz_plus, logeta_Ghz_plus, 1)
    sm = _sector_setup(ms, Ghz_minus, logeta_Ghz_minus, -1)
    K_p = np.exp(sp['logC'] - 16 * np.log(CSCALE) - SHIFT)
    K_m = np.exp(sm['logC'] - 16 * np.log(CSCALE) - SHIFT)

    import ml_dtypes
    bones = np.zeros((P, P), np.float32)
    for p1 in range(P):
        bones[p1, (p1 // 32) * 32:(p1 // 32) * 32 + 32] = 1.0
    bones = bones.astype(ml_dtypes.bfloat16)
    rmask = np.zeros((P, 31), np.float32)
    for j in range(31):
        rmask[:, j] = (np.arange(P) % 32 == j)

    res = dict(sp=sp, sm=sm, bones=bones, rmask=rmask, K_p=K_p, K_m=K_m)
    _setup_cache[key] = res
    return res


# ----------------------------------------------------------------------------
# device program
# ----------------------------------------------------------------------------
# S layout is c-major: Scat [P, 32c, 2ch, 4g] so that the matmul rhs for
# columns >= k+1 is a contiguous 2D tail slice (the ISA only allows 2D
# matmul operands). Stale values in columns < k+2 of T / rows < k+2 of S
# are never read by later steps (only the trailing block matters).

def _build_nc(nsteps=15, dump=False):
    global _nc_cache
    if _nc_cache is not None and nsteps == 15 and not dump:
        return _nc_cache
    nc = bacc.Bacc()
    scat_d = nc.dram_tensor("scat0", [P, 256], f32, kind="ExternalInput")
    bones_d = nc.dram_tensor("bones", [P, P], mybir.dt.bfloat16, kind="ExternalInput")
    rmask_d = nc.dram_tensor("rmask", [P, 31], f32, kind="ExternalInput")
    outp_d = nc.dram_tensor("outp", [P, 8 * nsteps], f32, kind="ExternalOutput")
    outt_d = nc.dram_tensor("outt", [P, 16], f32, kind="ExternalOutput")

    with tile.TileContext(nc) as tc:
        with tc.tile_pool(name="const", bufs=1) as cpool, \
             tc.tile_pool(name="state", bufs=1) as spool, \
             tc.tile_pool(name="temps", bufs=2) as tpool, \
             tc.tile_pool(name="psum", bufs=2, space="PSUM") as ppool:

            # dummy scalar op with no DMA deps: ACT_TABLE_LOAD overlaps DMAs
            warm = cpool.tile([P, 1], f32, tag="warm")
            nc.gpsimd.memset(warm[:], 0.0)
            nc.scalar.copy(warm[:], warm[:])

            bones = cpool.tile([P, P], mybir.dt.bfloat16, tag="bones")
            nc.sync.dma_start(bones[:], bones_d[:])
            rmask = cpool.tile([P, 31], f32, tag="rmask")
            nc.sync.dma_start(rmask[:], rmask_d[:])
            Scat = spool.tile([P, 32, 2, 4], f32, tag="Scat")
            Scat_f = Scat[:].rearrange("p c e g -> p (c e g)")
            nc.sync.dma_start(Scat_f[:, :128], scat_d[:, :128])
            nc.sync.dma_start(Scat_f[:, 128:], scat_d[:, 128:])

            selt = cpool.tile([P, nsteps, P], f32, tag="selt")

            def build_sel(si):
                nc.scalar.mul(selt[:, si, :], bones[:], rmask[:, 2 * si:2 * si + 1])

            build_sel(0)
            if nsteps > 1:
                build_sel(1)

            Scat2 = Scat[:].rearrange("p c e g -> p (c e g)")
            pivstore = spool.tile([P, nsteps, 2, 4], f32, tag="piv")

            for s in range(nsteps):
                k = 2 * s
                w1 = 31 - k   # broadcast columns k+1..31; apl c-index j = col k+1+j
                w2 = 30 - k

                apl = ppool.tile([P, 32, 2, 4], f32, tag="apl")
                nc.tensor.matmul(
                    apl[:].rearrange("p c e g -> p (c e g)")[:, :8 * w1],
                    selt[:, s, :], Scat2[:, 8 * (k + 1):], start=True, stop=True)

                # numerators first (only dep: matmul); |pi|^2 path runs
                # on scalar+vector in parallel and only gates the late Tscale
                cpair = Scat[:, k + 1]                            # [P, 2ch, 4g]
                nc.vector.tensor_copy(pivstore[:, s], apl[:, 0])
                pv = pivstore[:, s]
                pvr_b = pv[:, None, 0, :].broadcast_to([P, 2, 4])
                pvi_b = pv[:, None, 1, :].broadcast_to([P, 2, 4])
                P13 = tpool.tile([P, 2, 4], f32, tag="P13")
                nc.vector.tensor_mul(P13[:], cpair, pvr_b)
                P24 = tpool.tile([P, 2, 4], f32, tag="P24")
                nc.vector.tensor_mul(P24[:], cpair, pvi_b)
                # NN: [N2r | N2i | -N2r] (unscaled u2 numerators)
                NN = tpool.tile([P, 3, 4], f32, tag="NN")
                nc.vector.tensor_add(NN[:, 0], P13[:, 0], P24[:, 1])
                nc.vector.tensor_sub(NN[:, 1], P13[:, 1], P24[:, 0])
                nc.vector.tensor_scalar(out=NN[:, 2], in0=NN[:, 0], scalar1=-1.0,
                                        scalar2=None, op0=AOT.mult)

                if s + 2 < nsteps:
                    build_sel(s + 2)
                sq = tpool.tile([P, 2, 4], f32, tag="sq")
                nc.vector.tensor_mul(sq[:], pv, pv)
                den = tpool.tile([P, 4], f32, tag="den")
                nc.vector.tensor_add(den[:], sq[:, 0], sq[:, 1])
                rec = tpool.tile([P, 4], f32, tag="rec")
                nc.vector.reciprocal(rec[:], den[:])

                # T'' = N2 (x) (-row_k); T tiles are e-major so the stream
                # transpose runs on a contiguous view
                AR_b = apl[:, 1:w1, 0, None, :].transpose([0, 2, 3, 1]) \
                    .broadcast_to([P, 2, 4, w2])
                AI_b = apl[:, 1:w1, 1, None, :].transpose([0, 2, 3, 1]) \
                    .broadcast_to([P, 2, 4, w2])
                vA = NN[:, 0:2, :, None].broadcast_to([P, 2, 4, w2])
                vB = NN[:, 1:3, :, None].broadcast_to([P, 2, 4, w2])
                mA = tpool.tile([P, 2, 4, 32], f32, tag="mA")
                nc.vector.tensor_mul(mA[:, :, :, k + 2:], AR_b, vA)
                mB = tpool.tile([P, 2, 4, 32], f32, tag="mB")
                nc.vector.tensor_mul(mB[:, :, :, k + 2:], AI_b, vB)
                Tpp = tpool.tile([P, 2, 4, 32], f32, tag="Tpp")
                nc.vector.tensor_sub(Tpp[:, :, :, k + 2:], mB[:, :, :, k + 2:],
                                     mA[:, :, :, k + 2:])
                T = spool.tile([P, 2, 4, 32], f32, tag="T")
                nc.vector.tensor_mul(T[:, :, :, k + 2:], Tpp[:, :, :, k + 2:],
                                     rec[:, None, :, None].broadcast_to(
                                         [P, 2, 4, w2]))

                TT = spool.tile([P, 2, 4, 32], f32, tag="TT")
                nc.vector.transpose(TT[:].rearrange("p e g c -> p (e g c)"),
                                    T[:].rearrange("p e g c -> p (e g c)"))
                # S += T - T^T on trailing columns (S is c-major: view T/TT)
                Str = Scat[:, k + 2:]
                Tv = T[:].transpose([0, 3, 1, 2])
                TTv = TT[:].transpose([0, 3, 1, 2])
                nc.vector.scalar_tensor_tensor(
                    out=Str, in0=Tv[:, k + 2:], scalar=1.0, in1=Str,
                    op0=AOT.mult, op1=AOT.add)
                nc.vector.scalar_tensor_tensor(
                    out=Str, in0=TTv[:, k + 2:], scalar=-1.0, in1=Str,
                    op0=AOT.mult, op1=AOT.add)

            if dump:
                dump_d = nc.dram_tensor("dump_s", [P, 256], f32, kind="ExternalOutput")
                nc.sync.dma_start(dump_d[:], Scat2)

            nc.sync.dma_start(outp_d[:],
                              pivstore[:].rearrange("p s c g -> p (s c g)"))
            # final pivot S[30, 31] read host-side from the last two columns
            nc.sync.dma_start(outt_d[:].rearrange("p (c e g) -> p c e g",
                                                  c=2, e=2),
                              Scat[:, 30:])

    nc.compile()
    if nsteps == 15 and not dump:
        _nc_cache = nc
    return nc


# ----------------------------------------------------------------------------
# entry point
# ----------------------------------------------------------------------------

def kernel(x, s0, H1, H2):
    global LAST_RESULTS
    x64 = np.asarray(x, dtype=np.float64)
    s064 = np.asarray(s0, dtype=np.float64)
    H164 = np.asarray(H1, dtype=np.float64)
    H264 = np.asarray(H2, dtype=np.float64)
    B = x64.shape[0]
    assert B == 64 and x64.shape[1] == n

    st = _shared_setup(s064, H164, H264)
    nc = _build_nc()

    zz = x64 * np.roll(x64, -1, axis=1)
    sgn = np.sign(zz)

    zm = {}
    for plus, sd in ((True, st['sp']), (False, st['sm'])):
        zm[plus] = tuple((sd[kk] * CSCALE).astype(np.complex128)
                         for kk in ('Z11', 'Z10', 'Z01', 'Z00'))

    in_maps = []
    for c in range(NCORES):
        scat0 = np.zeros((P, 32, 2, 4), np.float32)   # c-major layout
        for mi in range(4):
            for gi in range(4):
                samp = c * 8 + mi * 2 + gi // 2
                plus = (gi % 2 == 0)
                Z11, Z10, Z01, Z00 = zm[plus]
                sig = sgn[samp].copy()
                sig[-1] *= -1.0 if plus else 1.0
                S = (Z00 - np.outer(sig, sig) * Z11
                     + 1j * sig[:, None] * Z10 + 1j * sig[None, :] * Z01)
                scat0[mi * 32:mi * 32 + 32, :, 0, gi] = S.real
                scat0[mi * 32:mi * 32 + 32, :, 1, gi] = S.imag
        in_maps.append(dict(scat0=scat0.reshape(P, 256),
                            bones=st['bones'], rmask=st['rmask']))

    trace = bool(int(os.environ.get("PFK_TRACE", "0")))
    res = run_bass_kernel_spmd(nc, in_maps, core_ids=list(range(NCORES)),
                               trace=trace)
    LAST_RESULTS = res

    out = np.zeros(B, dtype=np.complex128)
    xs_last = x64[:, -1] * s064[-1]
    for c in range(NCORES):
        op = np.asarray(res.results[c]["outp"], dtype=np.float64)
        ot = np.asarray(res.results[c]["outt"], dtype=np.float64)
        for mi in range(4):
            pm = op[32 * mi].reshape(15, 2, 4)         # [s, ch, g]
            pc = pm[:, 0, :] + 1j * pm[:, 1, :]        # [15, 4g]
            tl = ot[32 * mi + 30].reshape(2, 2, 4)     # [c(30,31), ch, g]
            fin = tl[1, 0, :] + 1j * tl[1, 1, :]       # [4g]
            pf = np.prod(pc, axis=0) * fin             # [4g]
            for j in range(2):
                samp = c * 8 + mi * 2 + j
                E_p = st['K_p'] * xs_last[samp] * pf[2 * j]
                E_m = st['K_m'] * pf[2 * j + 1]
                out[samp] = np.log(E_m + E_p) + SHIFT
    return out
